# revision 8
# baseline (speedup 1.0000x reference)
"""DepthwiseXCorr (SiamRPN-style) Trainium2 kernel.

reference:
  k = relu(BN(conv3x3(kernel)))   [B,256,7,7]  -> [B,256,5,5]
  s = relu(BN(conv3x3(search)))   [B,256,31,31]-> [B,256,29,29]
  out[b,c] = valid_xcorr(s[b,c], k[b,c])       -> [B,256,25,25]

Strategy (8 cores, data parallel over batch, 16 batches/core):
  - channel-on-partition layout, fp16 on chip, fp32 PSUM accumulation
  - both convs: 9 shifted matmuls accumulated in PSUM (BN scale folded into
    weights on host), ReLU+shift via ScalarE activation epilogue
  - depthwise xcorr: 25 per-partition-scalar FMA taps
    (scalar_tensor_tensor) split across VectorE and GpSimd
  - output written as fp16 [blk, b, c, 25x29(padded)]; host crops + casts
"""

import sys
import types
import contextlib
import numpy as np

C = 256
B = 128
NCORES = 8
BL = B // NCORES  # 16 local batches
BN_EPS = 1e-5

HS, WS = 31, 31          # search input
HK, WK = 7, 7            # kernel input
HSC, WSC = 29, 29        # search conv output (valid)
HKC, WKC = 5, 5          # kernel conv output (valid)
HO, WO = 25, 25          # xcorr output (valid)

SIMG = HS * WS           # 961
KIMG = HK * WK           # 49
YSIMG = HSC * WSC        # 841
NPAD_S = 64              # tail pad for shifted reads
NPAD_K = 64
YS_PAD = 8               # ys tile tail pad (max tap shift 4*29+4 + 725 = 845)
ACC_N = HO * WSC         # 725 = 25 rows x 29 cols (padded width)

# tap split across engines (tap index t = dy*5+dx)
DVE_TAPS = list(range(0, 25))
GP_TAPS = []


# ---------------------------------------------------------------------------
# environment fixups
# ---------------------------------------------------------------------------

_FIXED = False


def _install_env_fixups():
    global _FIXED
    if _FIXED:
        return
    _FIXED = True

    # -- 1. walrus in this image rejects >1 sync-wait on the Tile tail drain;
    #       spread waits over single-wait SP nops.
    import concourse.tile as tile_mod
    from concourse import mybir
    from concourse.vector_clock import ScopedClock

    def _patched_drain_and_barrier(self, tick_clock, wait_clock):
        nc = self.nc
        probe = nc.sync.nop(hint="drain_wait_spread", nofuse=True)
        wait_clock.add_sem_waits(
            probe.ins, ScopedClock({None: tick_clock.global_clock})
        )
        si = probe.ins.sync_info
        waits = list(si.on_wait) if si is not None and si.on_wait else []
        if len(waits) > 1:
            probe.ins.sync_info = mybir.SyncInfo(on_wait=waits[:1], on_update=[])
            for w in waits[1:]:
                n = nc.sync.nop(hint="drain_wait_spread", nofuse=True)
                n.ins.sync_info = mybir.SyncInfo(on_wait=[w], on_update=[])
        nc.sync.drain()
        nc.all_engine_barrier()
        assert self.sems is not None
        popped = nc._tile_sem_poison_stack.pop()
        assert popped is self._sem_poison
        nc.clear_and_free_semaphores(list(self.sems.allocated().values()))
        nc.all_engine_barrier()

    tile_mod.TileContext._drain_and_barrier = _patched_drain_and_barrier

    # -- 2. antenv.axon_hooks shim (NTFF profiling hook) so trace=True works.
    try:
        import antenv.axon_hooks  # noqa: F401
    except ImportError:
        import ctypes

        mod = types.ModuleType("antenv.axon_hooks")
        _holder = {}

        def set_axon_ntff_profile_hook(h):
            _holder["h"] = h

        def _default_hook():
            so_path = "/opt/axon/libaxon_pjrt.so"
            try:
                lib = ctypes.CDLL(so_path)
            except OSError:
                return None
            if not hasattr(lib, "axon_start_nrt_profile"):
                return None
            lib.axon_start_nrt_profile.argtypes = [
                ctypes.POINTER(ctypes.c_int64),
                ctypes.c_size_t,
            ]
            lib.axon_start_nrt_profile.restype = ctypes.c_int64
            lib.axon_stop_nrt_profile.argtypes = [ctypes.c_char_p]
            lib.axon_stop_nrt_profile.restype = ctypes.c_int64

            @contextlib.contextmanager
            def _hook(output_dir, device_ids):
                import jax

                jax.devices()
                if device_ids:
                    ids = (ctypes.c_int64 * len(device_ids))(*device_ids)
                    rc = lib.axon_start_nrt_profile(ids, len(device_ids))
                else:
                    rc = lib.axon_start_nrt_profile(None, 0)
                if rc != 0:
                    raise RuntimeError(f"axon_start_nrt_profile rc={rc}")
                try:
                    yield
                finally:
                    n = lib.axon_stop_nrt_profile(str(output_dir).encode())
                    if n < 0:
                        raise RuntimeError(f"axon_stop_nrt_profile rc={n}")

            return _hook

        def get_axon_ntff_profile_hook():
            if "h" in _holder:
                return _holder["h"]
            return _default_hook()

        mod.set_axon_ntff_profile_hook = set_axon_ntff_profile_hook
        mod.get_axon_ntff_profile_hook = get_axon_ntff_profile_hook
        sys.modules["antenv.axon_hooks"] = mod


# ---------------------------------------------------------------------------
# bass program
# ---------------------------------------------------------------------------

_PROGRAM = None


def _build_program():
    import concourse.tile as tile
    from concourse import bacc, mybir

    f16 = mybir.dt.float16
    f32 = mybir.dt.float32

    nc = bacc.Bacc("TRN2", target_bir_lowering=False, debug=False)

    xs_d = nc.dram_tensor("xs", [2, 128, BL * SIMG], f16, kind="ExternalInput")
    xk_d = nc.dram_tensor("xk", [2, 128, BL * KIMG], f16, kind="ExternalInput")
    ws_d = nc.dram_tensor("ws", [9, 2, 2, 128, 128], f16, kind="ExternalInput")
    wk_d = nc.dram_tensor("wk", [9, 2, 2, 128, 128], f16, kind="ExternalInput")
    shs_d = nc.dram_tensor("shs", [2, 128, 1], f32, kind="ExternalInput")
    shk_d = nc.dram_tensor("shk", [2, 128, 1], f32, kind="ExternalInput")
    y_d = nc.dram_tensor("y", [2, BL, 128, ACC_N], f16, kind="ExternalOutput")

    with tile.TileContext(nc) as tc:
        with (
            tc.tile_pool(name="wpool", bufs=1) as wpool,
            tc.tile_pool(name="xpool", bufs=1) as xpool,
            tc.tile_pool(name="shpool", bufs=1) as shpool,
            tc.tile_pool(name="ykpool", bufs=1) as ykpool,
            tc.tile_pool(name="yspool", bufs=4) as yspool,
            tc.tile_pool(name="accpool", bufs=4) as accpool,
            tc.tile_pool(name="pskc", bufs=2, space="PSUM") as pskc,
            tc.tile_pool(name="psa", bufs=2, space="PSUM") as psa,
            tc.tile_pool(name="psb", bufs=2, space="PSUM") as psb,
        ):
            # ---- static input loads ----
            ws_t = [[[wpool.tile([128, 128], f16, name=f"ws{t}_{ci}_{co}", tag=f"ws{t}_{ci}_{co}")
                      for co in range(2)] for ci in range(2)] for t in range(9)]
            wk_t = [[[wpool.tile([128, 128], f16, name=f"wk{t}_{ci}_{co}", tag=f"wk{t}_{ci}_{co}")
                      for co in range(2)] for ci in range(2)] for t in range(9)]
            for t in range(9):
                for ci in range(2):
                    for co in range(2):
                        nc.sync.dma_start(out=ws_t[t][ci][co][:],
                                          in_=ws_d.ap()[t, ci, co])
                        nc.sync.dma_start(out=wk_t[t][ci][co][:],
                                          in_=wk_d.ap()[t, ci, co])

            shs_t = [shpool.tile([128, 1], f32, name=f"shs{co}", tag=f"shs{co}") for co in range(2)]
            shk_t = [shpool.tile([128, 1], f32, name=f"shk{co}", tag=f"shk{co}") for co in range(2)]
            for co in range(2):
                nc.sync.dma_start(out=shs_t[co][:], in_=shs_d.ap()[co])
                nc.sync.dma_start(out=shk_t[co][:], in_=shk_d.ap()[co])

            xs_t = [xpool.tile([128, BL * SIMG + NPAD_S], f16, name=f"xs{ci}", tag=f"xs{ci}")
                    for ci in range(2)]
            xk_t = [xpool.tile([128, BL * KIMG + NPAD_K], f16, name=f"xk{ci}", tag=f"xk{ci}")
                    for ci in range(2)]
            for ci in range(2):
                nc.sync.dma_start(out=xs_t[ci][:, :BL * SIMG], in_=xs_d.ap()[ci])
                nc.gpsimd.memset(xs_t[ci][:, BL * SIMG:], 0.0)
                nc.sync.dma_start(out=xk_t[ci][:, :BL * KIMG], in_=xk_d.ap()[ci])
                nc.gpsimd.memset(xk_t[ci][:, BL * KIMG:], 0.0)

            # ---- kernel-branch conv: [2,128, BL*49] -> yk [2][128, BL*25] ----
            yk_t = [ykpool.tile([128, BL * 25], f32, name=f"yk{co}", tag=f"yk{co}") for co in range(2)]
            for co in range(2):
                for half in range(2):  # batches 0..7, 8..15
                    b0 = half * 8
                    ps = pskc.tile([128, 8 * 35], f32, name=f"pskc_{co}_{half}", tag="pskc")
                    mm = 0
                    for ci in range(2):
                        for t in range(9):
                            dh, dw = divmod(t, 3)
                            off = b0 * KIMG + dh * WK + dw
                            rhs = (xk_t[ci][:, off:off + 8 * KIMG]
                                   .rearrange("p (b q) -> p b q", q=KIMG)[:, :, :35])
                            nc.tensor.matmul(ps[:].rearrange("p (b q) -> p b q", q=35),
                                             ws_or(wk_t, t, ci, co),
                                             rhs,
                                             start=(mm == 0), stop=(mm == 17))
                            mm += 1
                    # relu(x + shift), compact 35 -> 25 cols per batch
                    nc.scalar.activation(
                        yk_t[co][:, b0 * 25:(b0 + 8) * 25]
                        .rearrange("p (b h q) -> p b h q", h=5, q=5),
                        ps[:].rearrange("p (b h q) -> p b h q", h=5, q=7)[:, :, :, :5],
                        mybir.ActivationFunctionType.Relu,
                        bias=shk_t[co][:, 0:1],
                    )

            # ---- per-batch: search conv + epilogue + xcorr ----
            # search conv output region per batch: n = h'*31 + w, h' 0..28, w 0..30
            # chunk A: rows 0..14 (465 cols), chunk B: rows 15..28 (434 cols)
            NA, NB = 15 * WS, 14 * WS
            for b in range(BL):
                for co in range(2):
                    ys = yspool.tile([128, YSIMG + YS_PAD], f16, name=f"ys_{b}_{co}", tag=f"ys{co}")
                    nc.gpsimd.memset(ys[:, YSIMG:], 0.0)
                    for (n0, cnt, r0, nrow) in ((0, NA, 0, 15), (NA, NB, 15, 14)):
                        pool = psa if n0 == 0 else psb
                        ps = pool.tile([128, cnt], f32, name=f"ps_{b}_{co}_{n0}", tag=f"ps{n0}")
                        mm = 0
                        for ci in range(2):
                            for t in range(9):
                                dh, dw = divmod(t, 3)
                                off = b * SIMG + n0 + dh * WS + dw
                                nc.tensor.matmul(ps[:], ws_or(ws_t, t, ci, co),
                                                 xs_t[ci][:, off:off + cnt],
                                                 start=(mm == 0), stop=(mm == 17))
                                mm += 1
                        nc.scalar.activation(
                            ys[:, r0 * WSC:(r0 + nrow) * WSC]
                            .rearrange("p (r q) -> p r q", q=WSC),
                            ps[:].rearrange("p (r q) -> p r q", q=WS)[:, :, :WSC],
                            mybir.ActivationFunctionType.Relu,
                            bias=shs_t[co][:, 0:1],
                        )

                    # xcorr: acc[p, n] = sum_t yk[p, b*25+t] * ys[p, n + dy*29+dx]
                    acc = accpool.tile([128, ACC_N], f16, name=f"acc_{b}_{co}", tag=f"acc{co}")
                    first = True
                    for t in DVE_TAPS + GP_TAPS:
                        dy, dx = divmod(t, 5)
                        off = dy * WSC + dx
                        sl = ys[:, off:off + ACC_N]
                        kcol = yk_t[co][:, b * 25 + t:b * 25 + t + 1]
                        if first:
                            nc.vector.tensor_scalar_mul(acc[:], sl, kcol)
                            first = False
                        else:
                            eng = nc.vector if t in DVE_TAPS else nc.gpsimd
                            eng.scalar_tensor_tensor(
                                acc[:], sl, kcol, acc[:],
                                mybir.AluOpType.mult, mybir.AluOpType.add)
                    nc.sync.dma_start(out=y_d.ap()[co, b], in_=acc[:])

    if not nc.is_finalized():
        nc.finalize()
    return nc


def ws_or(wlist, t, ci, co):
    return wlist[t][ci][co][:]


# ---------------------------------------------------------------------------
# host-side prep + entry point
# ---------------------------------------------------------------------------


def _fold_bn(w, gamma, beta, mean, var):
    # returns folded weight [C,C,3,3] and shift [C] (fp64 math)
    scale = gamma.astype(np.float64) / np.sqrt(var.astype(np.float64) + BN_EPS)
    shift = beta.astype(np.float64) - mean.astype(np.float64) * scale
    wf = w.astype(np.float64) * scale[:, None, None, None]
    return wf, shift


def _w_tiles(wf):
    # wf [C,C,3,3] -> [9, 2, 2, 128, 128] fp16 with lhsT[ci, co] layout
    out = np.empty((9, 2, 2, 128, 128), np.float16)
    for t in range(9):
        dh, dw = divmod(t, 3)
        m = wf[:, :, dh, dw]  # [co, ci]
        for ci in range(2):
            for co in range(2):
                out[t, ci, co] = (
                    m[co * 128:(co + 1) * 128, ci * 128:(ci + 1) * 128]
                    .T.astype(np.float16))
    return out


def _make_in_maps(inputs):
    wkf, shk = _fold_bn(inputs["w_kernel"], inputs["bn_k_gamma"],
                        inputs["bn_k_beta"], inputs["bn_k_mean"],
                        inputs["bn_k_var"])
    wsf, shs = _fold_bn(inputs["w_search"], inputs["bn_s_gamma"],
                        inputs["bn_s_beta"], inputs["bn_s_mean"],
                        inputs["bn_s_var"])
    wk_tiles = _w_tiles(wkf)
    ws_tiles = _w_tiles(wsf)
    shk_a = shk.astype(np.float32).reshape(2, 128, 1)
    shs_a = shs.astype(np.float32).reshape(2, 128, 1)

    # [B, C, H, W] -> per-core [2, 128, BL*IMG] fp16 channel-major
    def shard(x, img):
        x = np.asarray(x, np.float16).reshape(NCORES, BL, 2, 128, img)
        return np.ascontiguousarray(x.transpose(0, 2, 3, 1, 4)).reshape(
            NCORES, 2, 128, BL * img)

    xs_sh = shard(inputs["search"], SIMG)
    xk_sh = shard(inputs["kernel"], KIMG)

    return [
        dict(xs=xs_sh[i], xk=xk_sh[i], ws=ws_tiles, wk=wk_tiles,
             shs=shs_a, shk=shk_a)
        for i in range(NCORES)
    ]


def kernel(kernel, search, w_kernel, bn_k_gamma, bn_k_beta, bn_k_mean, bn_k_var,
           w_search, bn_s_gamma, bn_s_beta, bn_s_mean, bn_s_var):
    _install_env_fixups()
    from concourse.bass_utils import run_bass_kernel_spmd

    global _PROGRAM
    if _PROGRAM is None:
        _PROGRAM = _build_program()
    nc = _PROGRAM

    in_maps = _make_in_maps(dict(
        kernel=kernel, search=search, w_kernel=w_kernel,
        bn_k_gamma=bn_k_gamma, bn_k_beta=bn_k_beta, bn_k_mean=bn_k_mean,
        bn_k_var=bn_k_var, w_search=w_search, bn_s_gamma=bn_s_gamma,
        bn_s_beta=bn_s_beta, bn_s_mean=bn_s_mean, bn_s_var=bn_s_var))
    res = run_bass_kernel_spmd(nc, in_maps, list(range(NCORES)))

    out = np.empty((B, C, HO, WO), np.float32)
    for i in range(NCORES):
        y = res.results[i]["y"]  # [2, BL, 128, 725] fp16
        y = y.reshape(2, BL, 128, HO, WSC)[:, :, :, :, :WO].astype(np.float32)
        # -> [BL, 2*128, 25, 25]
        out[i * BL:(i + 1) * BL] = y.transpose(1, 0, 2, 3, 4).reshape(
            BL, C, HO, WO)
    return out


# revision 14
# speedup vs baseline: 1.2115x; 1.2115x over previous
"""DepthwiseXCorr (SiamRPN-style) Trainium2 kernel.

reference:
  k = relu(BN(conv3x3(kernel)))   [B,256,7,7]  -> [B,256,5,5]
  s = relu(BN(conv3x3(search)))   [B,256,31,31]-> [B,256,29,29]
  out[b,c] = valid_xcorr(s[b,c], k[b,c])       -> [B,256,25,25]

Strategy (8 cores, data parallel over batch, 16 batches/core):
  - channel-on-partition layout, fp16 on chip, fp32 PSUM accumulation
  - both convs: 9 shifted matmuls accumulated in PSUM (BN scale folded into
    weights on host), ReLU+shift via ScalarE activation epilogue
  - depthwise xcorr: 25 per-partition-scalar FMA taps
    (scalar_tensor_tensor) split across VectorE and GpSimd
  - output written as fp16 [blk, b, c, 25x29(padded)]; host crops + casts
"""

import sys
import types
import contextlib
import numpy as np

C = 256
B = 128
NCORES = 8
BL = B // NCORES  # 16 local batches
BN_EPS = 1e-5

HS, WS = 31, 31          # search input
HK, WK = 7, 7            # kernel input
HSC, WSC = 29, 29        # search conv output (valid)
HKC, WKC = 5, 5          # kernel conv output (valid)
HO, WO = 25, 25          # xcorr output (valid)

SIMG = HS * WS           # 961
KIMG = HK * WK           # 49
YSIMG = HSC * WSC        # 841
NPAD_S = 64              # tail pad for shifted reads
NPAD_K = 64
YS_PAD = 8               # ys tile tail pad (max tap shift 4*29+4 + 725 = 845)
ACC_N = HO * WSC         # 725 = 25 rows x 29 cols (padded width)

# tap split across engines (tap index t = dy*5+dx)
N_PE = 10   # taps 0..N_PE-1: TensorE full-diag matmuls accumulated in PSUM
N_DVE = 8   # next N_DVE taps: VectorE scalar_tensor_tensor (first seeds psum)
N_AG = 25 - N_PE - N_DVE  # rest: ScalarE product + GpSimd add


# ---------------------------------------------------------------------------
# environment fixups
# ---------------------------------------------------------------------------

_FIXED = False


def _install_env_fixups():
    global _FIXED
    if _FIXED:
        return
    _FIXED = True

    # -- 1. walrus in this image rejects >1 sync-wait on the Tile tail drain;
    #       spread waits over single-wait SP nops.
    import concourse.tile as tile_mod
    from concourse import mybir
    from concourse.vector_clock import ScopedClock

    def _patched_drain_and_barrier(self, tick_clock, wait_clock):
        nc = self.nc
        probe = nc.sync.nop(hint="drain_wait_spread", nofuse=True)
        wait_clock.add_sem_waits(
            probe.ins, ScopedClock({None: tick_clock.global_clock})
        )
        si = probe.ins.sync_info
        waits = list(si.on_wait) if si is not None and si.on_wait else []
        if len(waits) > 1:
            probe.ins.sync_info = mybir.SyncInfo(on_wait=waits[:1], on_update=[])
            for w in waits[1:]:
                n = nc.sync.nop(hint="drain_wait_spread", nofuse=True)
                n.ins.sync_info = mybir.SyncInfo(on_wait=[w], on_update=[])
        nc.sync.drain()
        nc.all_engine_barrier()
        assert self.sems is not None
        popped = nc._tile_sem_poison_stack.pop()
        assert popped is self._sem_poison
        nc.clear_and_free_semaphores(list(self.sems.allocated().values()))
        nc.all_engine_barrier()

    tile_mod.TileContext._drain_and_barrier = _patched_drain_and_barrier

    # -- 2. antenv.axon_hooks shim (NTFF profiling hook) so trace=True works.
    try:
        import antenv.axon_hooks  # noqa: F401
    except ImportError:
        import ctypes

        mod = types.ModuleType("antenv.axon_hooks")
        _holder = {}

        def set_axon_ntff_profile_hook(h):
            _holder["h"] = h

        def _default_hook():
            so_path = "/opt/axon/libaxon_pjrt.so"
            try:
                lib = ctypes.CDLL(so_path)
            except OSError:
                return None
            if not hasattr(lib, "axon_start_nrt_profile"):
                return None
            lib.axon_start_nrt_profile.argtypes = [
                ctypes.POINTER(ctypes.c_int64),
                ctypes.c_size_t,
            ]
            lib.axon_start_nrt_profile.restype = ctypes.c_int64
            lib.axon_stop_nrt_profile.argtypes = [ctypes.c_char_p]
            lib.axon_stop_nrt_profile.restype = ctypes.c_int64

            @contextlib.contextmanager
            def _hook(output_dir, device_ids):
                import jax

                jax.devices()
                if device_ids:
                    ids = (ctypes.c_int64 * len(device_ids))(*device_ids)
                    rc = lib.axon_start_nrt_profile(ids, len(device_ids))
                else:
                    rc = lib.axon_start_nrt_profile(None, 0)
                if rc != 0:
                    raise RuntimeError(f"axon_start_nrt_profile rc={rc}")
                try:
                    yield
                finally:
                    n = lib.axon_stop_nrt_profile(str(output_dir).encode())
                    if n < 0:
                        raise RuntimeError(f"axon_stop_nrt_profile rc={n}")

            return _hook

        def get_axon_ntff_profile_hook():
            if "h" in _holder:
                return _holder["h"]
            return _default_hook()

        mod.set_axon_ntff_profile_hook = set_axon_ntff_profile_hook
        mod.get_axon_ntff_profile_hook = get_axon_ntff_profile_hook
        sys.modules["antenv.axon_hooks"] = mod


# ---------------------------------------------------------------------------
# bass program
# ---------------------------------------------------------------------------

_PROGRAM = None


def _build_program():
    import concourse.tile as tile
    from concourse import bacc, mybir

    f16 = mybir.dt.float16
    f32 = mybir.dt.float32

    nc = bacc.Bacc("TRN2", target_bir_lowering=False, debug=False)

    xs_d = nc.dram_tensor("xs", [2, 128, BL * SIMG], f16, kind="ExternalInput")
    xk_d = nc.dram_tensor("xk", [2, 128, BL * KIMG], f16, kind="ExternalInput")
    ws_d = nc.dram_tensor("ws", [9, 2, 2, 128, 128], f16, kind="ExternalInput")
    wk_d = nc.dram_tensor("wk", [9, 2, 2, 128, 128], f16, kind="ExternalInput")
    shs_d = nc.dram_tensor("shs", [2, 128, 1], f32, kind="ExternalInput")
    shk_d = nc.dram_tensor("shk", [2, 128, 1], f32, kind="ExternalInput")
    id_d = nc.dram_tensor("ident", [128, 128], f16, kind="ExternalInput")
    y_d = nc.dram_tensor("y", [2, BL, 128, ACC_N], f16, kind="ExternalOutput")

    with tile.TileContext(nc) as tc:
        with (
            tc.tile_pool(name="wpool", bufs=1) as wpool,
            tc.tile_pool(name="xpool", bufs=1) as xpool,
            tc.tile_pool(name="shpool", bufs=1) as shpool,
            tc.tile_pool(name="ykpool", bufs=1) as ykpool,
            tc.tile_pool(name="yspool", bufs=4) as yspool,
            tc.tile_pool(name="accpool", bufs=4) as accpool,
            tc.tile_pool(name="dgpool", bufs=3) as dgpool,
            tc.tile_pool(name="tmppool", bufs=2) as tmppool,
            tc.tile_pool(name="psa", bufs=2, space="PSUM") as psa,
            tc.tile_pool(name="psb", bufs=2, space="PSUM") as psb,
            tc.tile_pool(name="psx", bufs=2, space="PSUM") as psx,
        ):
            # ---- static input loads ----
            ws_t = [[[wpool.tile([128, 128], f16, name=f"ws{t}_{ci}_{co}", tag=f"ws{t}_{ci}_{co}")
                      for co in range(2)] for ci in range(2)] for t in range(9)]
            wk_t = [[[wpool.tile([128, 128], f16, name=f"wk{t}_{ci}_{co}", tag=f"wk{t}_{ci}_{co}")
                      for co in range(2)] for ci in range(2)] for t in range(9)]
            for t in range(9):
                for ci in range(2):
                    for co in range(2):
                        nc.sync.dma_start(out=ws_t[t][ci][co][:],
                                          in_=ws_d.ap()[t, ci, co])
                        nc.sync.dma_start(out=wk_t[t][ci][co][:],
                                          in_=wk_d.ap()[t, ci, co])

            shs_t = [shpool.tile([128, 1], f32, name=f"shs{co}", tag=f"shs{co}") for co in range(2)]
            shk_t = [shpool.tile([128, 1], f32, name=f"shk{co}", tag=f"shk{co}") for co in range(2)]
            for co in range(2):
                nc.sync.dma_start(out=shs_t[co][:], in_=shs_d.ap()[co])
                nc.sync.dma_start(out=shk_t[co][:], in_=shk_d.ap()[co])

            xs_t = [xpool.tile([128, BL * SIMG + NPAD_S], f16, name=f"xs{ci}", tag=f"xs{ci}")
                    for ci in range(2)]
            xk_t = [xpool.tile([128, BL * KIMG + NPAD_K], f16, name=f"xk{ci}", tag=f"xk{ci}")
                    for ci in range(2)]
            for ci in range(2):
                nc.sync.dma_start(out=xs_t[ci][:, :BL * SIMG], in_=xs_d.ap()[ci])
                nc.gpsimd.memset(xs_t[ci][:, BL * SIMG:], 0.0)
                nc.sync.dma_start(out=xk_t[ci][:, :BL * KIMG], in_=xk_d.ap()[ci])
                nc.gpsimd.memset(xk_t[ci][:, BL * KIMG:], 0.0)

            ident = shpool.tile([128, 128], f16, name="ident", tag="ident")
            nc.sync.dma_start(out=ident[:], in_=id_d.ap())

            # ---- kernel-branch conv: [2,128, BL*49] -> yk [2][128, BL*25] ----
            yk_t = [ykpool.tile([128, BL * 25], f32, name=f"yk{co}", tag=f"yk{co}") for co in range(2)]
            for co in range(2):
                for half in range(2):  # batches 0..7, 8..15
                    b0 = half * 8
                    ps = psa.tile([128, 8 * 35], f32, name=f"pskc_{co}_{half}", tag="psA")
                    mm = 0
                    for ci in range(2):
                        for t in range(9):
                            dh, dw = divmod(t, 3)
                            off = b0 * KIMG + dh * WK + dw
                            rhs = (xk_t[ci][:, off:off + 8 * KIMG]
                                   .rearrange("p (b q) -> p b q", q=KIMG)[:, :, :35])
                            nc.tensor.matmul(ps[:].rearrange("p (b q) -> p b q", q=35),
                                             ws_or(wk_t, t, ci, co),
                                             rhs,
                                             start=(mm == 0), stop=(mm == 17))
                            mm += 1
                    # relu(x + shift), compact 35 -> 25 cols per batch
                    nc.scalar.activation(
                        yk_t[co][:, b0 * 25:(b0 + 8) * 25]
                        .rearrange("p (b h q) -> p b h q", h=5, q=5),
                        ps[:].rearrange("p (b h q) -> p b h q", h=5, q=7)[:, :, :, :5],
                        mybir.ActivationFunctionType.Relu,
                        bias=shk_t[co][:, 0:1],
                    )

            # ---- per-batch: search conv + epilogue + xcorr ----
            # search conv (exact 29-wide rows): chunk A rows 0..15 (464 cols),
            # chunk B rows 16..28 (377 cols)
            CHUNKS = ((0, 16), (16, 13))
            for b in range(BL):
                for co in range(2):
                    ys = yspool.tile([128, YSIMG + YS_PAD], f16, name=f"ys_{b}_{co}", tag=f"ys{co}")
                    nc.gpsimd.memset(ys[:, YSIMG:], 0.0)
                    for li, (r0, nrow) in enumerate(CHUNKS):
                        pool, tag = (psa, "psA") if li == 0 else (psb, "psB")
                        ps = pool.tile([128, nrow * WSC], f32, name=f"ps_{b}_{co}_{li}", tag=tag)
                        mm = 0
                        for ci in range(2):
                            for t in range(9):
                                dh, dw = divmod(t, 3)
                                base = b * SIMG + (r0 + dh) * WS + dw
                                rhs = (xs_t[ci][:, base:base + nrow * WS]
                                       .rearrange("p (r q) -> p r q", q=WS)[:, :, :WSC])
                                nc.tensor.matmul(
                                    ps[:].rearrange("p (r q) -> p r q", q=WSC),
                                    ws_or(ws_t, t, ci, co), rhs,
                                    start=(mm == 0), stop=(mm == 17))
                                mm += 1
                        nc.scalar.activation(
                            ys[:, r0 * WSC:(r0 + nrow) * WSC], ps[:],
                            mybir.ActivationFunctionType.Relu,
                            bias=shs_t[co][:, 0:1],
                        )

                    # xcorr: acc[p, n] = sum_t yk[p, b*25+t] * ys[p, n + dy*29+dx]
                    kofs = b * 25
                    taps = [(t,) + divmod(t, 5) for t in range(25)]

                    xps = None
                    if N_PE:
                        # diag matrices for PE taps, one DVE op
                        dg = dgpool.tile([128, N_PE * 128], f16,
                                         name=f"dg_{b}_{co}", tag=f"dg{co}")
                        nc.vector.tensor_tensor(
                            dg[:].rearrange("p (t j) -> p t j", j=128),
                            ident[:].rearrange("p (o j) -> p o j", o=1)
                            .broadcast_to((128, N_PE, 128)),
                            yk_t[co][:, kofs:kofs + N_PE]
                            .rearrange("p (t o) -> p t o", o=1)
                            .broadcast_to((128, N_PE, 128)),
                            mybir.AluOpType.mult)
                        xps = psx.tile([128, ACC_N], f32, name=f"xps_{b}_{co}", tag="psX")
                        for i in range(N_PE):
                            t, dy, dx = taps[i]
                            off = dy * WSC + dx
                            for (c0, cnt) in ((0, 512), (512, ACC_N - 512)):
                                nc.tensor.matmul(
                                    xps[:, c0:c0 + cnt],
                                    dg[:, i * 128:(i + 1) * 128],
                                    ys[:, off + c0:off + c0 + cnt],
                                    start=(i == 0), stop=(i == N_PE - 1))

                    acc = accpool.tile([128, ACC_N], f16, name=f"acc_{b}_{co}", tag=f"acc{co}")

                    # ScalarE products for the GpSimd taps (independent of acc)
                    tmps = []
                    for j in range(N_AG):
                        t, dy, dx = taps[N_PE + N_DVE + j]
                        off = dy * WSC + dx
                        tmp = tmppool.tile([128, ACC_N], f16,
                                           name=f"tmp{j}_{b}_{co}", tag=f"tmp{j}_{co}")
                        nc.scalar.activation(
                            tmp[:], ys[:, off:off + ACC_N],
                            mybir.ActivationFunctionType.Copy,
                            bias=0.0, scale=yk_t[co][:, kofs + t:kofs + t + 1])
                        tmps.append(tmp)

                    # VectorE chain (first seeds from PE psum partial)
                    for i in range(N_DVE):
                        t, dy, dx = taps[N_PE + i]
                        off = dy * WSC + dx
                        sl = ys[:, off:off + ACC_N]
                        kcol = yk_t[co][:, kofs + t:kofs + t + 1]
                        if i == 0 and xps is not None:
                            nc.vector.scalar_tensor_tensor(
                                acc[:], sl, kcol, xps[:],
                                mybir.AluOpType.mult, mybir.AluOpType.add)
                        elif i == 0:
                            nc.vector.tensor_scalar_mul(acc[:], sl, kcol)
                        else:
                            nc.vector.scalar_tensor_tensor(
                                acc[:], sl, kcol, acc[:],
                                mybir.AluOpType.mult, mybir.AluOpType.add)

                    # GpSimd accumulates the ScalarE products
                    for tmp in tmps:
                        nc.gpsimd.tensor_tensor(acc[:], tmp[:], acc[:],
                                                mybir.AluOpType.add)

                    nc.sync.dma_start(out=y_d.ap()[co, b], in_=acc[:])

    if not nc.is_finalized():
        nc.finalize()
    return nc


def ws_or(wlist, t, ci, co):
    return wlist[t][ci][co][:]


# ---------------------------------------------------------------------------
# host-side prep + entry point
# ---------------------------------------------------------------------------


def _fold_bn(w, gamma, beta, mean, var):
    # returns folded weight [C,C,3,3] and shift [C] (fp64 math)
    scale = gamma.astype(np.float64) / np.sqrt(var.astype(np.float64) + BN_EPS)
    shift = beta.astype(np.float64) - mean.astype(np.float64) * scale
    wf = w.astype(np.float64) * scale[:, None, None, None]
    return wf, shift


def _w_tiles(wf):
    # wf [C,C,3,3] -> [9, 2, 2, 128, 128] fp16 with lhsT[ci, co] layout
    out = np.empty((9, 2, 2, 128, 128), np.float16)
    for t in range(9):
        dh, dw = divmod(t, 3)
        m = wf[:, :, dh, dw]  # [co, ci]
        for ci in range(2):
            for co in range(2):
                out[t, ci, co] = (
                    m[co * 128:(co + 1) * 128, ci * 128:(ci + 1) * 128]
                    .T.astype(np.float16))
    return out


def _make_in_maps(inputs):
    wkf, shk = _fold_bn(inputs["w_kernel"], inputs["bn_k_gamma"],
                        inputs["bn_k_beta"], inputs["bn_k_mean"],
                        inputs["bn_k_var"])
    wsf, shs = _fold_bn(inputs["w_search"], inputs["bn_s_gamma"],
                        inputs["bn_s_beta"], inputs["bn_s_mean"],
                        inputs["bn_s_var"])
    wk_tiles = _w_tiles(wkf)
    ws_tiles = _w_tiles(wsf)
    shk_a = shk.astype(np.float32).reshape(2, 128, 1)
    shs_a = shs.astype(np.float32).reshape(2, 128, 1)

    # [B, C, H, W] -> per-core [2, 128, BL*IMG] fp16 channel-major
    def shard(x, img):
        x = np.asarray(x, np.float16).reshape(NCORES, BL, 2, 128, img)
        return np.ascontiguousarray(x.transpose(0, 2, 3, 1, 4)).reshape(
            NCORES, 2, 128, BL * img)

    xs_sh = shard(inputs["search"], SIMG)
    xk_sh = shard(inputs["kernel"], KIMG)

    ident = np.eye(128, dtype=np.float16)
    return [
        dict(xs=xs_sh[i], xk=xk_sh[i], ws=ws_tiles, wk=wk_tiles,
             shs=shs_a, shk=shk_a, ident=ident)
        for i in range(NCORES)
    ]


def kernel(kernel, search, w_kernel, bn_k_gamma, bn_k_beta, bn_k_mean, bn_k_var,
           w_search, bn_s_gamma, bn_s_beta, bn_s_mean, bn_s_var):
    _install_env_fixups()
    from concourse.bass_utils import run_bass_kernel_spmd

    global _PROGRAM
    if _PROGRAM is None:
        _PROGRAM = _build_program()
    nc = _PROGRAM

    in_maps = _make_in_maps(dict(
        kernel=kernel, search=search, w_kernel=w_kernel,
        bn_k_gamma=bn_k_gamma, bn_k_beta=bn_k_beta, bn_k_mean=bn_k_mean,
        bn_k_var=bn_k_var, w_search=w_search, bn_s_gamma=bn_s_gamma,
        bn_s_beta=bn_s_beta, bn_s_mean=bn_s_mean, bn_s_var=bn_s_var))
    res = run_bass_kernel_spmd(nc, in_maps, list(range(NCORES)))

    out = np.empty((B, C, HO, WO), np.float32)
    for i in range(NCORES):
        y = res.results[i]["y"]  # [2, BL, 128, 725] fp16
        y = y.reshape(2, BL, 128, HO, WSC)[:, :, :, :, :WO].astype(np.float32)
        # -> [BL, 2*128, 25, 25]
        out[i * BL:(i + 1) * BL] = y.transpose(1, 0, 2, 3, 4).reshape(
            BL, C, HO, WO)
    return out


# revision 15
# speedup vs baseline: 1.7256x; 1.4244x over previous
"""DepthwiseXCorr (SiamRPN-style) Trainium2 kernel.

reference:
  k = relu(BN(conv3x3(kernel)))   [B,256,7,7]  -> [B,256,5,5]
  s = relu(BN(conv3x3(search)))   [B,256,31,31]-> [B,256,29,29]
  out[b,c] = valid_xcorr(s[b,c], k[b,c])       -> [B,256,25,25]

Strategy (8 cores, data parallel over batch, 16 batches/core):
  - channel-on-partition layout, fp16 on chip, fp32 PSUM accumulation
  - both convs: 9 shifted matmuls accumulated in PSUM (BN scale folded into
    weights on host), ReLU+shift via ScalarE activation epilogue
  - depthwise xcorr: 25 per-partition-scalar FMA taps
    (scalar_tensor_tensor) split across VectorE and GpSimd
  - output written as fp16 [blk, b, c, 25x29(padded)]; host crops + casts
"""

import sys
import types
import contextlib
import numpy as np

C = 256
B = 128
NCORES = 8
BL = B // NCORES  # 16 local batches
BN_EPS = 1e-5

HS, WS = 31, 31          # search input
HK, WK = 7, 7            # kernel input
HSC, WSC = 29, 29        # search conv output (valid)
HKC, WKC = 5, 5          # kernel conv output (valid)
HO, WO = 25, 25          # xcorr output (valid)

SIMG = HS * WS           # 961
KIMG = HK * WK           # 49
YSIMG = HSC * WSC        # 841
NPAD_S = 64              # tail pad for shifted reads
NPAD_K = 64
YS_PAD = 8               # ys tile tail pad (max tap shift 4*29+4 + 725 = 845)
ACC_N = HO * WSC         # 725 = 25 rows x 29 cols (padded width)

# tap split across engines (tap index t = dy*5+dx)
N_PE = 12   # taps 0..N_PE-1: TensorE full-diag matmuls accumulated in PSUM
N_DVE = 4   # next N_DVE taps: VectorE scalar_tensor_tensor (first seeds psum)
N_AD = 25 - N_PE - N_DVE  # rest: ScalarE product + VectorE tensor_tensor add


# ---------------------------------------------------------------------------
# environment fixups
# ---------------------------------------------------------------------------

_FIXED = False


def _install_env_fixups():
    global _FIXED
    if _FIXED:
        return
    _FIXED = True

    # -- 1. walrus in this image rejects >1 sync-wait on the Tile tail drain;
    #       spread waits over single-wait SP nops.
    import concourse.tile as tile_mod
    from concourse import mybir
    from concourse.vector_clock import ScopedClock

    def _patched_drain_and_barrier(self, tick_clock, wait_clock):
        nc = self.nc
        probe = nc.sync.nop(hint="drain_wait_spread", nofuse=True)
        wait_clock.add_sem_waits(
            probe.ins, ScopedClock({None: tick_clock.global_clock})
        )
        si = probe.ins.sync_info
        waits = list(si.on_wait) if si is not None and si.on_wait else []
        if len(waits) > 1:
            probe.ins.sync_info = mybir.SyncInfo(on_wait=waits[:1], on_update=[])
            for w in waits[1:]:
                n = nc.sync.nop(hint="drain_wait_spread", nofuse=True)
                n.ins.sync_info = mybir.SyncInfo(on_wait=[w], on_update=[])
        nc.sync.drain()
        nc.all_engine_barrier()
        assert self.sems is not None
        popped = nc._tile_sem_poison_stack.pop()
        assert popped is self._sem_poison
        nc.clear_and_free_semaphores(list(self.sems.allocated().values()))
        nc.all_engine_barrier()

    tile_mod.TileContext._drain_and_barrier = _patched_drain_and_barrier

    # -- 2. antenv.axon_hooks shim (NTFF profiling hook) so trace=True works.
    try:
        import antenv.axon_hooks  # noqa: F401
    except ImportError:
        import ctypes

        mod = types.ModuleType("antenv.axon_hooks")
        _holder = {}

        def set_axon_ntff_profile_hook(h):
            _holder["h"] = h

        def _default_hook():
            so_path = "/opt/axon/libaxon_pjrt.so"
            try:
                lib = ctypes.CDLL(so_path)
            except OSError:
                return None
            if not hasattr(lib, "axon_start_nrt_profile"):
                return None
            lib.axon_start_nrt_profile.argtypes = [
                ctypes.POINTER(ctypes.c_int64),
                ctypes.c_size_t,
            ]
            lib.axon_start_nrt_profile.restype = ctypes.c_int64
            lib.axon_stop_nrt_profile.argtypes = [ctypes.c_char_p]
            lib.axon_stop_nrt_profile.restype = ctypes.c_int64

            @contextlib.contextmanager
            def _hook(output_dir, device_ids):
                import jax

                jax.devices()
                if device_ids:
                    ids = (ctypes.c_int64 * len(device_ids))(*device_ids)
                    rc = lib.axon_start_nrt_profile(ids, len(device_ids))
                else:
                    rc = lib.axon_start_nrt_profile(None, 0)
                if rc != 0:
                    raise RuntimeError(f"axon_start_nrt_profile rc={rc}")
                try:
                    yield
                finally:
                    n = lib.axon_stop_nrt_profile(str(output_dir).encode())
                    if n < 0:
                        raise RuntimeError(f"axon_stop_nrt_profile rc={n}")

            return _hook

        def get_axon_ntff_profile_hook():
            if "h" in _holder:
                return _holder["h"]
            return _default_hook()

        mod.set_axon_ntff_profile_hook = set_axon_ntff_profile_hook
        mod.get_axon_ntff_profile_hook = get_axon_ntff_profile_hook
        sys.modules["antenv.axon_hooks"] = mod


# ---------------------------------------------------------------------------
# bass program
# ---------------------------------------------------------------------------

_PROGRAM = None


def _build_program():
    import concourse.tile as tile
    from concourse import bacc, mybir

    f16 = mybir.dt.float16
    f32 = mybir.dt.float32

    nc = bacc.Bacc("TRN2", target_bir_lowering=False, debug=False)

    xs_d = nc.dram_tensor("xs", [2, 128, BL * SIMG], f16, kind="ExternalInput")
    xk_d = nc.dram_tensor("xk", [2, 128, BL * KIMG], f16, kind="ExternalInput")
    ws_d = nc.dram_tensor("ws", [9, 2, 2, 128, 128], f16, kind="ExternalInput")
    wk_d = nc.dram_tensor("wk", [9, 2, 2, 128, 128], f16, kind="ExternalInput")
    shs_d = nc.dram_tensor("shs", [2, 128, 1], f32, kind="ExternalInput")
    shk_d = nc.dram_tensor("shk", [2, 128, 1], f32, kind="ExternalInput")
    id_d = nc.dram_tensor("ident", [128, 128], f16, kind="ExternalInput")
    y_d = nc.dram_tensor("y", [2, BL, 128, ACC_N], f16, kind="ExternalOutput")

    with tile.TileContext(nc) as tc:
        with (
            tc.tile_pool(name="wpool", bufs=1) as wpool,
            tc.tile_pool(name="xpool", bufs=1) as xpool,
            tc.tile_pool(name="shpool", bufs=1) as shpool,
            tc.tile_pool(name="ykpool", bufs=1) as ykpool,
            tc.tile_pool(name="yspool", bufs=4) as yspool,
            tc.tile_pool(name="accpool", bufs=4) as accpool,
            tc.tile_pool(name="dgpool", bufs=3) as dgpool,
            tc.tile_pool(name="tmppool", bufs=2) as tmppool,
            tc.tile_pool(name="psa", bufs=2, space="PSUM") as psa,
            tc.tile_pool(name="psb", bufs=2, space="PSUM") as psb,
            tc.tile_pool(name="psx", bufs=2, space="PSUM") as psx,
        ):
            # ---- static input loads ----
            ws_t = [[[wpool.tile([128, 128], f16, name=f"ws{t}_{ci}_{co}", tag=f"ws{t}_{ci}_{co}")
                      for co in range(2)] for ci in range(2)] for t in range(9)]
            wk_t = [[[wpool.tile([128, 128], f16, name=f"wk{t}_{ci}_{co}", tag=f"wk{t}_{ci}_{co}")
                      for co in range(2)] for ci in range(2)] for t in range(9)]
            for t in range(9):
                for ci in range(2):
                    for co in range(2):
                        nc.sync.dma_start(out=ws_t[t][ci][co][:],
                                          in_=ws_d.ap()[t, ci, co])
                        nc.sync.dma_start(out=wk_t[t][ci][co][:],
                                          in_=wk_d.ap()[t, ci, co])

            shs_t = [shpool.tile([128, 1], f32, name=f"shs{co}", tag=f"shs{co}") for co in range(2)]
            shk_t = [shpool.tile([128, 1], f32, name=f"shk{co}", tag=f"shk{co}") for co in range(2)]
            for co in range(2):
                nc.sync.dma_start(out=shs_t[co][:], in_=shs_d.ap()[co])
                nc.sync.dma_start(out=shk_t[co][:], in_=shk_d.ap()[co])

            xs_t = [xpool.tile([128, BL * SIMG + NPAD_S], f16, name=f"xs{ci}", tag=f"xs{ci}")
                    for ci in range(2)]
            xk_t = [xpool.tile([128, BL * KIMG + NPAD_K], f16, name=f"xk{ci}", tag=f"xk{ci}")
                    for ci in range(2)]
            for ci in range(2):
                nc.sync.dma_start(out=xs_t[ci][:, :BL * SIMG], in_=xs_d.ap()[ci])
                nc.gpsimd.memset(xs_t[ci][:, BL * SIMG:], 0.0)
                nc.sync.dma_start(out=xk_t[ci][:, :BL * KIMG], in_=xk_d.ap()[ci])
                nc.gpsimd.memset(xk_t[ci][:, BL * KIMG:], 0.0)

            ident = shpool.tile([128, 128], f16, name="ident", tag="ident")
            nc.sync.dma_start(out=ident[:], in_=id_d.ap())

            # ---- kernel-branch conv: [2,128, BL*49] -> yk [2][128, BL*25] ----
            yk_t = [ykpool.tile([128, BL * 25], f32, name=f"yk{co}", tag=f"yk{co}") for co in range(2)]
            for co in range(2):
                for half in range(2):  # batches 0..7, 8..15
                    b0 = half * 8
                    ps = psa.tile([128, 8 * 35], f32, name=f"pskc_{co}_{half}", tag="psA")
                    mm = 0
                    for ci in range(2):
                        for t in range(9):
                            dh, dw = divmod(t, 3)
                            off = b0 * KIMG + dh * WK + dw
                            rhs = (xk_t[ci][:, off:off + 8 * KIMG]
                                   .rearrange("p (b q) -> p b q", q=KIMG)[:, :, :35])
                            nc.tensor.matmul(ps[:].rearrange("p (b q) -> p b q", q=35),
                                             ws_or(wk_t, t, ci, co),
                                             rhs,
                                             start=(mm == 0), stop=(mm == 17))
                            mm += 1
                    # relu(x + shift), compact 35 -> 25 cols per batch
                    nc.scalar.activation(
                        yk_t[co][:, b0 * 25:(b0 + 8) * 25]
                        .rearrange("p (b h q) -> p b h q", h=5, q=5),
                        ps[:].rearrange("p (b h q) -> p b h q", h=5, q=7)[:, :, :, :5],
                        mybir.ActivationFunctionType.Relu,
                        bias=shk_t[co][:, 0:1],
                    )

            # ---- per-batch: search conv + epilogue + xcorr ----
            # search conv (exact 29-wide rows): chunk A rows 0..15 (464 cols),
            # chunk B rows 16..28 (377 cols)
            CHUNKS = ((0, 16), (16, 13))
            for b in range(BL):
                for co in range(2):
                    ys = yspool.tile([128, YSIMG + YS_PAD], f16, name=f"ys_{b}_{co}", tag=f"ys{co}")
                    nc.gpsimd.memset(ys[:, YSIMG:], 0.0)
                    for li, (r0, nrow) in enumerate(CHUNKS):
                        pool, tag = (psa, "psA") if li == 0 else (psb, "psB")
                        ps = pool.tile([128, nrow * WSC], f32, name=f"ps_{b}_{co}_{li}", tag=tag)
                        mm = 0
                        for ci in range(2):
                            for t in range(9):
                                dh, dw = divmod(t, 3)
                                base = b * SIMG + (r0 + dh) * WS + dw
                                rhs = (xs_t[ci][:, base:base + nrow * WS]
                                       .rearrange("p (r q) -> p r q", q=WS)[:, :, :WSC])
                                nc.tensor.matmul(
                                    ps[:].rearrange("p (r q) -> p r q", q=WSC),
                                    ws_or(ws_t, t, ci, co), rhs,
                                    start=(mm == 0), stop=(mm == 17))
                                mm += 1
                        nc.scalar.activation(
                            ys[:, r0 * WSC:(r0 + nrow) * WSC], ps[:],
                            mybir.ActivationFunctionType.Relu,
                            bias=shs_t[co][:, 0:1],
                        )

                    # xcorr: acc[p, n] = sum_t yk[p, b*25+t] * ys[p, n + dy*29+dx]
                    kofs = b * 25
                    taps = [(t,) + divmod(t, 5) for t in range(25)]

                    xps = None
                    if N_PE:
                        # diag matrices for PE taps, one DVE op
                        dg = dgpool.tile([128, N_PE * 128], f16,
                                         name=f"dg_{b}_{co}", tag=f"dg{co}")
                        nc.vector.tensor_tensor(
                            dg[:].rearrange("p (t j) -> p t j", j=128),
                            ident[:].rearrange("p (o j) -> p o j", o=1)
                            .broadcast_to((128, N_PE, 128)),
                            yk_t[co][:, kofs:kofs + N_PE]
                            .rearrange("p (t o) -> p t o", o=1)
                            .broadcast_to((128, N_PE, 128)),
                            mybir.AluOpType.mult)
                        xps = psx.tile([128, ACC_N], f32, name=f"xps_{b}_{co}", tag="psX")
                        for i in range(N_PE):
                            t, dy, dx = taps[i]
                            off = dy * WSC + dx
                            for (c0, cnt) in ((0, 512), (512, ACC_N - 512)):
                                nc.tensor.matmul(
                                    xps[:, c0:c0 + cnt],
                                    dg[:, i * 128:(i + 1) * 128],
                                    ys[:, off + c0:off + c0 + cnt],
                                    start=(i == 0), stop=(i == N_PE - 1))

                    acc = accpool.tile([128, ACC_N], f16, name=f"acc_{b}_{co}", tag=f"acc{co}")

                    # ScalarE products, added into acc by VectorE tensor_tensor
                    tmps = []
                    for j in range(N_AD):
                        t, dy, dx = taps[N_PE + N_DVE + j]
                        off = dy * WSC + dx
                        tmp = tmppool.tile([128, ACC_N], f16,
                                           name=f"tmp{j}_{b}_{co}", tag=f"tmp{j}_{co}")
                        nc.scalar.activation(
                            tmp[:], ys[:, off:off + ACC_N],
                            mybir.ActivationFunctionType.Copy,
                            bias=0.0, scale=yk_t[co][:, kofs + t:kofs + t + 1])
                        tmps.append(tmp)

                    # VectorE chain (first seeds from PE psum partial)
                    for i in range(N_DVE):
                        t, dy, dx = taps[N_PE + i]
                        off = dy * WSC + dx
                        sl = ys[:, off:off + ACC_N]
                        kcol = yk_t[co][:, kofs + t:kofs + t + 1]
                        if i == 0 and xps is not None:
                            nc.vector.scalar_tensor_tensor(
                                acc[:], sl, kcol, xps[:],
                                mybir.AluOpType.mult, mybir.AluOpType.add)
                        elif i == 0:
                            nc.vector.tensor_scalar_mul(acc[:], sl, kcol)
                        else:
                            nc.vector.scalar_tensor_tensor(
                                acc[:], sl, kcol, acc[:],
                                mybir.AluOpType.mult, mybir.AluOpType.add)

                    # VectorE accumulates the ScalarE products
                    for tmp in tmps:
                        nc.vector.tensor_tensor(acc[:], tmp[:], acc[:],
                                                mybir.AluOpType.add)

                    nc.sync.dma_start(out=y_d.ap()[co, b], in_=acc[:])

    if not nc.is_finalized():
        nc.finalize()
    return nc


def ws_or(wlist, t, ci, co):
    return wlist[t][ci][co][:]


# ---------------------------------------------------------------------------
# host-side prep + entry point
# ---------------------------------------------------------------------------


def _fold_bn(w, gamma, beta, mean, var):
    # returns folded weight [C,C,3,3] and shift [C] (fp64 math)
    scale = gamma.astype(np.float64) / np.sqrt(var.astype(np.float64) + BN_EPS)
    shift = beta.astype(np.float64) - mean.astype(np.float64) * scale
    wf = w.astype(np.float64) * scale[:, None, None, None]
    return wf, shift


def _w_tiles(wf):
    # wf [C,C,3,3] -> [9, 2, 2, 128, 128] fp16 with lhsT[ci, co] layout
    out = np.empty((9, 2, 2, 128, 128), np.float16)
    for t in range(9):
        dh, dw = divmod(t, 3)
        m = wf[:, :, dh, dw]  # [co, ci]
        for ci in range(2):
            for co in range(2):
                out[t, ci, co] = (
                    m[co * 128:(co + 1) * 128, ci * 128:(ci + 1) * 128]
                    .T.astype(np.float16))
    return out


def _make_in_maps(inputs):
    wkf, shk = _fold_bn(inputs["w_kernel"], inputs["bn_k_gamma"],
                        inputs["bn_k_beta"], inputs["bn_k_mean"],
                        inputs["bn_k_var"])
    wsf, shs = _fold_bn(inputs["w_search"], inputs["bn_s_gamma"],
                        inputs["bn_s_beta"], inputs["bn_s_mean"],
                        inputs["bn_s_var"])
    wk_tiles = _w_tiles(wkf)
    ws_tiles = _w_tiles(wsf)
    shk_a = shk.astype(np.float32).reshape(2, 128, 1)
    shs_a = shs.astype(np.float32).reshape(2, 128, 1)

    # [B, C, H, W] -> per-core [2, 128, BL*IMG] fp16 channel-major
    def shard(x, img):
        x = np.asarray(x, np.float16).reshape(NCORES, BL, 2, 128, img)
        return np.ascontiguousarray(x.transpose(0, 2, 3, 1, 4)).reshape(
            NCORES, 2, 128, BL * img)

    xs_sh = shard(inputs["search"], SIMG)
    xk_sh = shard(inputs["kernel"], KIMG)

    ident = np.eye(128, dtype=np.float16)
    return [
        dict(xs=xs_sh[i], xk=xk_sh[i], ws=ws_tiles, wk=wk_tiles,
             shs=shs_a, shk=shk_a, ident=ident)
        for i in range(NCORES)
    ]


def kernel(kernel, search, w_kernel, bn_k_gamma, bn_k_beta, bn_k_mean, bn_k_var,
           w_search, bn_s_gamma, bn_s_beta, bn_s_mean, bn_s_var):
    _install_env_fixups()
    from concourse.bass_utils import run_bass_kernel_spmd

    global _PROGRAM
    if _PROGRAM is None:
        _PROGRAM = _build_program()
    nc = _PROGRAM

    in_maps = _make_in_maps(dict(
        kernel=kernel, search=search, w_kernel=w_kernel,
        bn_k_gamma=bn_k_gamma, bn_k_beta=bn_k_beta, bn_k_mean=bn_k_mean,
        bn_k_var=bn_k_var, w_search=w_search, bn_s_gamma=bn_s_gamma,
        bn_s_beta=bn_s_beta, bn_s_mean=bn_s_mean, bn_s_var=bn_s_var))
    res = run_bass_kernel_spmd(nc, in_maps, list(range(NCORES)))

    out = np.empty((B, C, HO, WO), np.float32)
    for i in range(NCORES):
        y = res.results[i]["y"]  # [2, BL, 128, 725] fp16
        y = y.reshape(2, BL, 128, HO, WSC)[:, :, :, :, :WO].astype(np.float32)
        # -> [BL, 2*128, 25, 25]
        out[i * BL:(i + 1) * BL] = y.transpose(1, 0, 2, 3, 4).reshape(
            BL, C, HO, WO)
    return out


# revision 18
# speedup vs baseline: 1.7318x; 1.0036x over previous
"""DepthwiseXCorr (SiamRPN-style) Trainium2 kernel.

reference:
  k = relu(BN(conv3x3(kernel)))   [B,256,7,7]  -> [B,256,5,5]
  s = relu(BN(conv3x3(search)))   [B,256,31,31]-> [B,256,29,29]
  out[b,c] = valid_xcorr(s[b,c], k[b,c])       -> [B,256,25,25]

Strategy (8 cores, data parallel over batch, 16 batches/core):
  - channel-on-partition layout, fp16 on chip, fp32 PSUM accumulation
  - both convs: 9 shifted matmuls accumulated in PSUM (BN scale folded into
    weights on host), ReLU+shift via ScalarE activation epilogue
  - depthwise xcorr: 25 per-partition-scalar FMA taps
    (scalar_tensor_tensor) split across VectorE and GpSimd
  - output written as fp16 [blk, b, c, 25x29(padded)]; host crops + casts
"""

import sys
import types
import contextlib
import numpy as np

C = 256
B = 128
NCORES = 8
BL = B // NCORES  # 16 local batches
BN_EPS = 1e-5

HS, WS = 31, 31          # search input
HK, WK = 7, 7            # kernel input
HSC, WSC = 29, 29        # search conv output (valid)
HKC, WKC = 5, 5          # kernel conv output (valid)
HO, WO = 25, 25          # xcorr output (valid)

SIMG = HS * WS           # 961
KIMG = HK * WK           # 49
YSIMG = HSC * WSC        # 841
NPAD_S = 64              # tail pad for shifted reads
NPAD_K = 64
YS_PAD = 8               # ys tile tail pad (max tap shift 4*29+4 + 725 = 845)
ACC_N = HO * WSC         # 725 = 25 rows x 29 cols (padded width)

# tap split across engines (tap index t = dy*5+dx)
N_PE = 14   # taps 0..N_PE-1: TensorE full-diag matmuls accumulated in PSUM
N_DVE = 4   # next N_DVE taps: VectorE scalar_tensor_tensor, in-place on acc
N_AD = 25 - N_PE - N_DVE  # rest: ScalarE product + VectorE tensor_tensor add


# ---------------------------------------------------------------------------
# environment fixups
# ---------------------------------------------------------------------------

_FIXED = False


def _install_env_fixups():
    global _FIXED
    if _FIXED:
        return
    _FIXED = True

    # -- 1. walrus in this image rejects >1 sync-wait on the Tile tail drain;
    #       spread waits over single-wait SP nops.
    import concourse.tile as tile_mod
    from concourse import mybir
    from concourse.vector_clock import ScopedClock

    def _patched_drain_and_barrier(self, tick_clock, wait_clock):
        nc = self.nc
        probe = nc.sync.nop(hint="drain_wait_spread", nofuse=True)
        wait_clock.add_sem_waits(
            probe.ins, ScopedClock({None: tick_clock.global_clock})
        )
        si = probe.ins.sync_info
        waits = list(si.on_wait) if si is not None and si.on_wait else []
        if len(waits) > 1:
            probe.ins.sync_info = mybir.SyncInfo(on_wait=waits[:1], on_update=[])
            for w in waits[1:]:
                n = nc.sync.nop(hint="drain_wait_spread", nofuse=True)
                n.ins.sync_info = mybir.SyncInfo(on_wait=[w], on_update=[])
        nc.sync.drain()
        nc.all_engine_barrier()
        assert self.sems is not None
        popped = nc._tile_sem_poison_stack.pop()
        assert popped is self._sem_poison
        nc.clear_and_free_semaphores(list(self.sems.allocated().values()))
        nc.all_engine_barrier()

    tile_mod.TileContext._drain_and_barrier = _patched_drain_and_barrier

    # -- 1b. optionally re-enable walrus LDWEIGHTS scheduling optimization
    #        (hides weight loads under matmul streaming).
    import os as _os

    if _os.environ.get("KERNEL_LDW_OPT", "1") == "1":
        import concourse.bass_utils as _bu

        _orig_bvo = _bu.bir_verify_and_optimise

        def _bvo(*args, **kwargs):
            import subprocess as _sp

            orig_run = _bu.run_command

            def run_command(cmd, **kw):
                cmd = [c.replace("--enable-ldw-opt=false", "--enable-ldw-opt=true")
                       if isinstance(c, str) else c for c in cmd]
                return orig_run(cmd, **kw)

            _bu.run_command = run_command
            try:
                return _orig_bvo(*args, **kwargs)
            finally:
                _bu.run_command = orig_run

        _bu.bir_verify_and_optimise = _bvo

    # -- 2. antenv.axon_hooks shim (NTFF profiling hook) so trace=True works.
    try:
        import antenv.axon_hooks  # noqa: F401
    except ImportError:
        import ctypes

        mod = types.ModuleType("antenv.axon_hooks")
        _holder = {}

        def set_axon_ntff_profile_hook(h):
            _holder["h"] = h

        def _default_hook():
            so_path = "/opt/axon/libaxon_pjrt.so"
            try:
                lib = ctypes.CDLL(so_path)
            except OSError:
                return None
            if not hasattr(lib, "axon_start_nrt_profile"):
                return None
            lib.axon_start_nrt_profile.argtypes = [
                ctypes.POINTER(ctypes.c_int64),
                ctypes.c_size_t,
            ]
            lib.axon_start_nrt_profile.restype = ctypes.c_int64
            lib.axon_stop_nrt_profile.argtypes = [ctypes.c_char_p]
            lib.axon_stop_nrt_profile.restype = ctypes.c_int64

            @contextlib.contextmanager
            def _hook(output_dir, device_ids):
                import jax

                jax.devices()
                if device_ids:
                    ids = (ctypes.c_int64 * len(device_ids))(*device_ids)
                    rc = lib.axon_start_nrt_profile(ids, len(device_ids))
                else:
                    rc = lib.axon_start_nrt_profile(None, 0)
                if rc != 0:
                    raise RuntimeError(f"axon_start_nrt_profile rc={rc}")
                try:
                    yield
                finally:
                    n = lib.axon_stop_nrt_profile(str(output_dir).encode())
                    if n < 0:
                        raise RuntimeError(f"axon_stop_nrt_profile rc={n}")

            return _hook

        def get_axon_ntff_profile_hook():
            if "h" in _holder:
                return _holder["h"]
            return _default_hook()

        mod.set_axon_ntff_profile_hook = set_axon_ntff_profile_hook
        mod.get_axon_ntff_profile_hook = get_axon_ntff_profile_hook
        sys.modules["antenv.axon_hooks"] = mod


# ---------------------------------------------------------------------------
# bass program
# ---------------------------------------------------------------------------

_PROGRAM = None


def _build_program():
    import concourse.tile as tile
    from concourse import bacc, mybir

    f16 = mybir.dt.float16
    f32 = mybir.dt.float32

    nc = bacc.Bacc("TRN2", target_bir_lowering=False, debug=False)

    xs_d = nc.dram_tensor("xs", [2, 128, BL * SIMG], f16, kind="ExternalInput")
    xk_d = nc.dram_tensor("xk", [2, 128, BL * KIMG], f16, kind="ExternalInput")
    ws_d = nc.dram_tensor("ws", [9, 2, 2, 128, 128], f16, kind="ExternalInput")
    wk_d = nc.dram_tensor("wk", [9, 2, 2, 128, 128], f16, kind="ExternalInput")
    shs_d = nc.dram_tensor("shs", [2, 128, 1], f32, kind="ExternalInput")
    shk_d = nc.dram_tensor("shk", [2, 128, 1], f32, kind="ExternalInput")
    id_d = nc.dram_tensor("ident", [128, 128], f16, kind="ExternalInput")
    y_d = nc.dram_tensor("y", [2, BL, 128, ACC_N], f16, kind="ExternalOutput")

    with tile.TileContext(nc) as tc:
        with (
            tc.tile_pool(name="wpool", bufs=1) as wpool,
            tc.tile_pool(name="xpool", bufs=1) as xpool,
            tc.tile_pool(name="shpool", bufs=1) as shpool,
            tc.tile_pool(name="ykpool", bufs=1) as ykpool,
            tc.tile_pool(name="yspool", bufs=5) as yspool,
            tc.tile_pool(name="accpool", bufs=6) as accpool,
            tc.tile_pool(name="dgpool", bufs=4) as dgpool,
            tc.tile_pool(name="tmppool", bufs=2) as tmppool,
            tc.tile_pool(name="psa", bufs=2, space="PSUM") as psa,
            tc.tile_pool(name="psb", bufs=2, space="PSUM") as psb,
            tc.tile_pool(name="psx", bufs=2, space="PSUM") as psx,
        ):
            # ---- static input loads ----
            ws_t = [[[wpool.tile([128, 128], f16, name=f"ws{t}_{ci}_{co}", tag=f"ws{t}_{ci}_{co}")
                      for co in range(2)] for ci in range(2)] for t in range(9)]
            wk_t = [[[wpool.tile([128, 128], f16, name=f"wk{t}_{ci}_{co}", tag=f"wk{t}_{ci}_{co}")
                      for co in range(2)] for ci in range(2)] for t in range(9)]
            for t in range(9):
                for ci in range(2):
                    for co in range(2):
                        nc.sync.dma_start(out=ws_t[t][ci][co][:],
                                          in_=ws_d.ap()[t, ci, co])
                        nc.sync.dma_start(out=wk_t[t][ci][co][:],
                                          in_=wk_d.ap()[t, ci, co])

            shs_t = [shpool.tile([128, 1], f32, name=f"shs{co}", tag=f"shs{co}") for co in range(2)]
            shk_t = [shpool.tile([128, 1], f32, name=f"shk{co}", tag=f"shk{co}") for co in range(2)]
            for co in range(2):
                nc.sync.dma_start(out=shs_t[co][:], in_=shs_d.ap()[co])
                nc.sync.dma_start(out=shk_t[co][:], in_=shk_d.ap()[co])

            xs_t = [xpool.tile([128, BL * SIMG + NPAD_S], f16, name=f"xs{ci}", tag=f"xs{ci}")
                    for ci in range(2)]
            xk_t = [xpool.tile([128, BL * KIMG + NPAD_K], f16, name=f"xk{ci}", tag=f"xk{ci}")
                    for ci in range(2)]
            for ci in range(2):
                nc.sync.dma_start(out=xs_t[ci][:, :BL * SIMG], in_=xs_d.ap()[ci])
                nc.gpsimd.memset(xs_t[ci][:, BL * SIMG:], 0.0)
                nc.sync.dma_start(out=xk_t[ci][:, :BL * KIMG], in_=xk_d.ap()[ci])
                nc.gpsimd.memset(xk_t[ci][:, BL * KIMG:], 0.0)

            ident = shpool.tile([128, 128], f16, name="ident", tag="ident")
            nc.sync.dma_start(out=ident[:], in_=id_d.ap())

            # ---- kernel-branch conv: [2,128, BL*49] -> yk [2][128, BL*25] ----
            yk_t = [ykpool.tile([128, BL * 25], f32, name=f"yk{co}", tag=f"yk{co}") for co in range(2)]
            for co in range(2):
                for half in range(2):  # batches 0..7, 8..15
                    b0 = half * 8
                    ps = psa.tile([128, 8 * 35], f32, name=f"pskc_{co}_{half}", tag="psA")
                    mm = 0
                    for ci in range(2):
                        for t in range(9):
                            dh, dw = divmod(t, 3)
                            off = b0 * KIMG + dh * WK + dw
                            rhs = (xk_t[ci][:, off:off + 8 * KIMG]
                                   .rearrange("p (b q) -> p b q", q=KIMG)[:, :, :35])
                            nc.tensor.matmul(ps[:].rearrange("p (b q) -> p b q", q=35),
                                             ws_or(wk_t, t, ci, co),
                                             rhs,
                                             start=(mm == 0), stop=(mm == 17))
                            mm += 1
                    # relu(x + shift), compact 35 -> 25 cols per batch
                    nc.scalar.activation(
                        yk_t[co][:, b0 * 25:(b0 + 8) * 25]
                        .rearrange("p (b h q) -> p b h q", h=5, q=5),
                        ps[:].rearrange("p (b h q) -> p b h q", h=5, q=7)[:, :, :, :5],
                        mybir.ActivationFunctionType.Relu,
                        bias=shk_t[co][:, 0:1],
                    )

            # ---- per-batch: search conv + epilogue + xcorr ----
            # search conv (exact 29-wide rows): chunk A rows 0..15 (464 cols),
            # chunk B rows 16..28 (377 cols)
            CHUNKS = ((0, 16), (16, 13))
            for b in range(BL):
                for co in range(2):
                    ys = yspool.tile([128, YSIMG + YS_PAD], f16, name=f"ys_{b}_{co}", tag=f"ys{co}")
                    nc.gpsimd.memset(ys[:, YSIMG:], 0.0)
                    for li, (r0, nrow) in enumerate(CHUNKS):
                        pool, tag = (psa, "psA") if li == 0 else (psb, "psB")
                        ps = pool.tile([128, nrow * WSC], f32, name=f"ps_{b}_{co}_{li}", tag=tag)
                        mm = 0
                        for ci in range(2):
                            for t in range(9):
                                dh, dw = divmod(t, 3)
                                base = b * SIMG + (r0 + dh) * WS + dw
                                rhs = (xs_t[ci][:, base:base + nrow * WS]
                                       .rearrange("p (r q) -> p r q", q=WS)[:, :, :WSC])
                                nc.tensor.matmul(
                                    ps[:].rearrange("p (r q) -> p r q", q=WSC),
                                    ws_or(ws_t, t, ci, co), rhs,
                                    start=(mm == 0), stop=(mm == 17))
                                mm += 1
                        nc.scalar.activation(
                            ys[:, r0 * WSC:(r0 + nrow) * WSC], ps[:],
                            mybir.ActivationFunctionType.Relu,
                            bias=shs_t[co][:, 0:1],
                        )

                    # xcorr: acc[p, n] = sum_t yk[p, b*25+t] * ys[p, n + dy*29+dx]
                    kofs = b * 25
                    taps = [(t,) + divmod(t, 5) for t in range(25)]

                    xps = None
                    if N_PE:
                        # diag matrices for PE taps, one DVE op
                        dg = dgpool.tile([128, N_PE * 128], f16,
                                         name=f"dg_{b}_{co}", tag=f"dg{co}")
                        nc.vector.tensor_tensor(
                            dg[:].rearrange("p (t j) -> p t j", j=128),
                            ident[:].rearrange("p (o j) -> p o j", o=1)
                            .broadcast_to((128, N_PE, 128)),
                            yk_t[co][:, kofs:kofs + N_PE]
                            .rearrange("p (t o) -> p t o", o=1)
                            .broadcast_to((128, N_PE, 128)),
                            mybir.AluOpType.mult)
                        xps = psx.tile([128, ACC_N], f32, name=f"xps_{b}_{co}", tag="psX")
                        for i in range(N_PE):
                            t, dy, dx = taps[i]
                            off = dy * WSC + dx
                            for (c0, cnt) in ((0, 512), (512, ACC_N - 512)):
                                nc.tensor.matmul(
                                    xps[:, c0:c0 + cnt],
                                    dg[:, i * 128:(i + 1) * 128],
                                    ys[:, off + c0:off + c0 + cnt],
                                    start=(i == 0), stop=(i == N_PE - 1))

                    acc = accpool.tile([128, ACC_N], f16, name=f"acc_{b}_{co}", tag=f"acc{co}")

                    # ScalarE products, added into acc by VectorE tensor_tensor
                    tmps = []
                    for j in range(N_AD):
                        t, dy, dx = taps[N_PE + N_DVE + j]
                        off = dy * WSC + dx
                        tmp = tmppool.tile([128, ACC_N], f16,
                                           name=f"tmp{j}_{b}_{co}", tag=f"tmp{j}_{co}")
                        nc.scalar.activation(
                            tmp[:], ys[:, off:off + ACC_N],
                            mybir.ActivationFunctionType.Copy,
                            bias=0.0, scale=yk_t[co][:, kofs + t:kofs + t + 1])
                        tmps.append(tmp)

                    # seed acc from PE psum partial via ScalarE, then
                    # VectorE chain in place
                    if xps is not None:
                        nc.scalar.activation(acc[:], xps[:],
                                             mybir.ActivationFunctionType.Copy)
                    for i in range(N_DVE):
                        t, dy, dx = taps[N_PE + i]
                        off = dy * WSC + dx
                        sl = ys[:, off:off + ACC_N]
                        kcol = yk_t[co][:, kofs + t:kofs + t + 1]
                        if i == 0 and xps is None:
                            nc.vector.tensor_scalar_mul(acc[:], sl, kcol)
                        else:
                            nc.vector.scalar_tensor_tensor(
                                acc[:], sl, kcol, acc[:],
                                mybir.AluOpType.mult, mybir.AluOpType.add)

                    # VectorE accumulates the ScalarE products
                    for tmp in tmps:
                        nc.vector.tensor_tensor(acc[:], tmp[:], acc[:],
                                                mybir.AluOpType.add)

                    nc.sync.dma_start(out=y_d.ap()[co, b], in_=acc[:])

    if not nc.is_finalized():
        nc.finalize()
    return nc


def ws_or(wlist, t, ci, co):
    return wlist[t][ci][co][:]


# ---------------------------------------------------------------------------
# host-side prep + entry point
# ---------------------------------------------------------------------------


def _fold_bn(w, gamma, beta, mean, var):
    # returns folded weight [C,C,3,3] and shift [C] (fp64 math)
    scale = gamma.astype(np.float64) / np.sqrt(var.astype(np.float64) + BN_EPS)
    shift = beta.astype(np.float64) - mean.astype(np.float64) * scale
    wf = w.astype(np.float64) * scale[:, None, None, None]
    return wf, shift


def _w_tiles(wf):
    # wf [C,C,3,3] -> [9, 2, 2, 128, 128] fp16 with lhsT[ci, co] layout
    out = np.empty((9, 2, 2, 128, 128), np.float16)
    for t in range(9):
        dh, dw = divmod(t, 3)
        m = wf[:, :, dh, dw]  # [co, ci]
        for ci in range(2):
            for co in range(2):
                out[t, ci, co] = (
                    m[co * 128:(co + 1) * 128, ci * 128:(ci + 1) * 128]
                    .T.astype(np.float16))
    return out


def _make_in_maps(inputs):
    wkf, shk = _fold_bn(inputs["w_kernel"], inputs["bn_k_gamma"],
                        inputs["bn_k_beta"], inputs["bn_k_mean"],
                        inputs["bn_k_var"])
    wsf, shs = _fold_bn(inputs["w_search"], inputs["bn_s_gamma"],
                        inputs["bn_s_beta"], inputs["bn_s_mean"],
                        inputs["bn_s_var"])
    wk_tiles = _w_tiles(wkf)
    ws_tiles = _w_tiles(wsf)
    shk_a = shk.astype(np.float32).reshape(2, 128, 1)
    shs_a = shs.astype(np.float32).reshape(2, 128, 1)

    # [B, C, H, W] -> per-core [2, 128, BL*IMG] fp16 channel-major
    def shard(x, img):
        x = np.asarray(x, np.float16).reshape(NCORES, BL, 2, 128, img)
        return np.ascontiguousarray(x.transpose(0, 2, 3, 1, 4)).reshape(
            NCORES, 2, 128, BL * img)

    xs_sh = shard(inputs["search"], SIMG)
    xk_sh = shard(inputs["kernel"], KIMG)

    ident = np.eye(128, dtype=np.float16)
    return [
        dict(xs=xs_sh[i], xk=xk_sh[i], ws=ws_tiles, wk=wk_tiles,
             shs=shs_a, shk=shk_a, ident=ident)
        for i in range(NCORES)
    ]


def kernel(kernel, search, w_kernel, bn_k_gamma, bn_k_beta, bn_k_mean, bn_k_var,
           w_search, bn_s_gamma, bn_s_beta, bn_s_mean, bn_s_var):
    _install_env_fixups()
    from concourse.bass_utils import run_bass_kernel_spmd

    global _PROGRAM
    if _PROGRAM is None:
        _PROGRAM = _build_program()
    nc = _PROGRAM

    in_maps = _make_in_maps(dict(
        kernel=kernel, search=search, w_kernel=w_kernel,
        bn_k_gamma=bn_k_gamma, bn_k_beta=bn_k_beta, bn_k_mean=bn_k_mean,
        bn_k_var=bn_k_var, w_search=w_search, bn_s_gamma=bn_s_gamma,
        bn_s_beta=bn_s_beta, bn_s_mean=bn_s_mean, bn_s_var=bn_s_var))
    res = run_bass_kernel_spmd(nc, in_maps, list(range(NCORES)))

    out = np.empty((B, C, HO, WO), np.float32)
    for i in range(NCORES):
        y = res.results[i]["y"]  # [2, BL, 128, 725] fp16
        y = y.reshape(2, BL, 128, HO, WSC)[:, :, :, :, :WO].astype(np.float32)
        # -> [BL, 2*128, 25, 25]
        out[i * BL:(i + 1) * BL] = y.transpose(1, 0, 2, 3, 4).reshape(
            BL, C, HO, WO)
    return out


# revision 19
# speedup vs baseline: 1.9466x; 1.1240x over previous
"""DepthwiseXCorr (SiamRPN-style) Trainium2 kernel.

reference:
  k = relu(BN(conv3x3(kernel)))   [B,256,7,7]  -> [B,256,5,5]
  s = relu(BN(conv3x3(search)))   [B,256,31,31]-> [B,256,29,29]
  out[b,c] = valid_xcorr(s[b,c], k[b,c])       -> [B,256,25,25]

Strategy (8 cores, data parallel over batch, 16 batches/core):
  - channel-on-partition layout, fp16 on chip, fp32 PSUM accumulation
  - both convs: 9 shifted matmuls accumulated in PSUM (BN scale folded into
    weights on host), ReLU+shift via ScalarE activation epilogue
  - depthwise xcorr: 25 per-partition-scalar FMA taps
    (scalar_tensor_tensor) split across VectorE and GpSimd
  - output written as fp16 [blk, b, c, 25x29(padded)]; host crops + casts
"""

import sys
import types
import contextlib
import numpy as np

C = 256
B = 128
NCORES = 8
BL = B // NCORES  # 16 local batches
BN_EPS = 1e-5

HS, WS = 31, 31          # search input
HK, WK = 7, 7            # kernel input
HSC, WSC = 29, 29        # search conv output (valid)
HKC, WKC = 5, 5          # kernel conv output (valid)
HO, WO = 25, 25          # xcorr output (valid)

SIMG = HS * WS           # 961
KIMG = HK * WK           # 49
YSIMG = HSC * WSC        # 841
NPAD_S = 64              # tail pad for shifted reads
NPAD_K = 64
YS_PAD = 8               # ys tile tail pad (max tap shift 4*29+4 + 725 = 845)
ACC_N = HO * WSC         # 725 = 25 rows x 29 cols (padded width)

# tap split across engines (tap index t = dy*5+dx)
N_PE = 13   # taps 0..N_PE-1: TensorE full-diag matmuls accumulated in PSUM
N_DVE = 4   # next N_DVE taps: VectorE scalar_tensor_tensor, in-place on acc
N_AD = 25 - N_PE - N_DVE  # rest: ScalarE product + VectorE tensor_tensor add


# ---------------------------------------------------------------------------
# environment fixups
# ---------------------------------------------------------------------------

_FIXED = False


def _install_env_fixups():
    global _FIXED
    if _FIXED:
        return
    _FIXED = True

    # -- 1. walrus in this image rejects >1 sync-wait on the Tile tail drain;
    #       spread waits over single-wait SP nops.
    import concourse.tile as tile_mod
    from concourse import mybir
    from concourse.vector_clock import ScopedClock

    def _patched_drain_and_barrier(self, tick_clock, wait_clock):
        nc = self.nc
        probe = nc.sync.nop(hint="drain_wait_spread", nofuse=True)
        wait_clock.add_sem_waits(
            probe.ins, ScopedClock({None: tick_clock.global_clock})
        )
        si = probe.ins.sync_info
        waits = list(si.on_wait) if si is not None and si.on_wait else []
        if len(waits) > 1:
            probe.ins.sync_info = mybir.SyncInfo(on_wait=waits[:1], on_update=[])
            for w in waits[1:]:
                n = nc.sync.nop(hint="drain_wait_spread", nofuse=True)
                n.ins.sync_info = mybir.SyncInfo(on_wait=[w], on_update=[])
        nc.sync.drain()
        nc.all_engine_barrier()
        assert self.sems is not None
        popped = nc._tile_sem_poison_stack.pop()
        assert popped is self._sem_poison
        nc.clear_and_free_semaphores(list(self.sems.allocated().values()))
        nc.all_engine_barrier()

    tile_mod.TileContext._drain_and_barrier = _patched_drain_and_barrier

    # -- 1b. optionally re-enable walrus LDWEIGHTS scheduling optimization
    #        (hides weight loads under matmul streaming).
    import os as _os

    if _os.environ.get("KERNEL_LDW_OPT", "1") == "1":
        import concourse.bass_utils as _bu

        _orig_bvo = _bu.bir_verify_and_optimise

        def _bvo(*args, **kwargs):
            import subprocess as _sp

            orig_run = _bu.run_command

            def run_command(cmd, **kw):
                cmd = [c.replace("--enable-ldw-opt=false", "--enable-ldw-opt=true")
                       if isinstance(c, str) else c for c in cmd]
                return orig_run(cmd, **kw)

            _bu.run_command = run_command
            try:
                return _orig_bvo(*args, **kwargs)
            finally:
                _bu.run_command = orig_run

        _bu.bir_verify_and_optimise = _bvo

    # -- 2. antenv.axon_hooks shim (NTFF profiling hook) so trace=True works.
    try:
        import antenv.axon_hooks  # noqa: F401
    except ImportError:
        import ctypes

        mod = types.ModuleType("antenv.axon_hooks")
        _holder = {}

        def set_axon_ntff_profile_hook(h):
            _holder["h"] = h

        def _default_hook():
            so_path = "/opt/axon/libaxon_pjrt.so"
            try:
                lib = ctypes.CDLL(so_path)
            except OSError:
                return None
            if not hasattr(lib, "axon_start_nrt_profile"):
                return None
            lib.axon_start_nrt_profile.argtypes = [
                ctypes.POINTER(ctypes.c_int64),
                ctypes.c_size_t,
            ]
            lib.axon_start_nrt_profile.restype = ctypes.c_int64
            lib.axon_stop_nrt_profile.argtypes = [ctypes.c_char_p]
            lib.axon_stop_nrt_profile.restype = ctypes.c_int64

            @contextlib.contextmanager
            def _hook(output_dir, device_ids):
                import jax

                jax.devices()
                if device_ids:
                    ids = (ctypes.c_int64 * len(device_ids))(*device_ids)
                    rc = lib.axon_start_nrt_profile(ids, len(device_ids))
                else:
                    rc = lib.axon_start_nrt_profile(None, 0)
                if rc != 0:
                    raise RuntimeError(f"axon_start_nrt_profile rc={rc}")
                try:
                    yield
                finally:
                    n = lib.axon_stop_nrt_profile(str(output_dir).encode())
                    if n < 0:
                        raise RuntimeError(f"axon_stop_nrt_profile rc={n}")

            return _hook

        def get_axon_ntff_profile_hook():
            if "h" in _holder:
                return _holder["h"]
            return _default_hook()

        mod.set_axon_ntff_profile_hook = set_axon_ntff_profile_hook
        mod.get_axon_ntff_profile_hook = get_axon_ntff_profile_hook
        sys.modules["antenv.axon_hooks"] = mod


# ---------------------------------------------------------------------------
# bass program
# ---------------------------------------------------------------------------

_PROGRAM = None


def _build_program():
    import concourse.tile as tile
    from concourse import bacc, mybir

    f16 = mybir.dt.float16
    f32 = mybir.dt.float32

    nc = bacc.Bacc("TRN2", target_bir_lowering=False, debug=False)

    xs_d = nc.dram_tensor("xs", [2, 128, BL * SIMG], f16, kind="ExternalInput")
    xk_d = nc.dram_tensor("xk", [2, 128, BL * KIMG], f16, kind="ExternalInput")
    ws_d = nc.dram_tensor("ws", [2, 128, 9 * 2 * 128], f16, kind="ExternalInput")
    wk_d = nc.dram_tensor("wk", [2, 128, 9 * 2 * 128], f16, kind="ExternalInput")
    shs_d = nc.dram_tensor("shs", [2, 128, 1], f32, kind="ExternalInput")
    shk_d = nc.dram_tensor("shk", [2, 128, 1], f32, kind="ExternalInput")
    id_d = nc.dram_tensor("ident", [128, 128], f16, kind="ExternalInput")
    y_d = nc.dram_tensor("y", [2, BL, 128, ACC_N], f16, kind="ExternalOutput")

    with tile.TileContext(nc) as tc:
        with (
            tc.tile_pool(name="wpool", bufs=1) as wpool,
            tc.tile_pool(name="xpool", bufs=1) as xpool,
            tc.tile_pool(name="shpool", bufs=1) as shpool,
            tc.tile_pool(name="ykpool", bufs=1) as ykpool,
            tc.tile_pool(name="yspool", bufs=5) as yspool,
            tc.tile_pool(name="accpool", bufs=6) as accpool,
            tc.tile_pool(name="dgpool", bufs=4) as dgpool,
            tc.tile_pool(name="tmppool", bufs=2) as tmppool,
            tc.tile_pool(name="psa", bufs=2, space="PSUM") as psa,
            tc.tile_pool(name="psb", bufs=2, space="PSUM") as psb,
            tc.tile_pool(name="psx", bufs=2, space="PSUM") as psx,
        ):
            # ---- static input loads (weights packed: 1 DMA per tensor/block) ----
            ws_t = [wpool.tile([128, 9 * 2 * 128], f16, name=f"wsp{ci}", tag=f"wsp{ci}")
                    for ci in range(2)]
            wk_t = [wpool.tile([128, 9 * 2 * 128], f16, name=f"wkp{ci}", tag=f"wkp{ci}")
                    for ci in range(2)]
            for ci in range(2):
                nc.sync.dma_start(out=ws_t[ci][:], in_=ws_d.ap()[ci])
                nc.sync.dma_start(out=wk_t[ci][:], in_=wk_d.ap()[ci])

            shs_t = [shpool.tile([128, 1], f32, name=f"shs{co}", tag=f"shs{co}") for co in range(2)]
            shk_t = [shpool.tile([128, 1], f32, name=f"shk{co}", tag=f"shk{co}") for co in range(2)]
            for co in range(2):
                nc.sync.dma_start(out=shs_t[co][:], in_=shs_d.ap()[co])
                nc.sync.dma_start(out=shk_t[co][:], in_=shk_d.ap()[co])

            xs_t = [xpool.tile([128, BL * SIMG + NPAD_S], f16, name=f"xs{ci}", tag=f"xs{ci}")
                    for ci in range(2)]
            xk_t = [xpool.tile([128, BL * KIMG + NPAD_K], f16, name=f"xk{ci}", tag=f"xk{ci}")
                    for ci in range(2)]
            for ci in range(2):
                nc.sync.dma_start(out=xs_t[ci][:, :BL * SIMG], in_=xs_d.ap()[ci])
                nc.gpsimd.memset(xs_t[ci][:, BL * SIMG:], 0.0)
                nc.sync.dma_start(out=xk_t[ci][:, :BL * KIMG], in_=xk_d.ap()[ci])
                nc.gpsimd.memset(xk_t[ci][:, BL * KIMG:], 0.0)

            ident = shpool.tile([128, 128], f16, name="ident", tag="ident")
            nc.sync.dma_start(out=ident[:], in_=id_d.ap())

            # ---- kernel-branch conv: [2,128, BL*49] -> yk [2][128, BL*25] ----
            yk_t = [ykpool.tile([128, BL * 25], f32, name=f"yk{co}", tag=f"yk{co}") for co in range(2)]
            for co in range(2):
                for half in range(2):  # batches 0..7, 8..15
                    b0 = half * 8
                    ps = psa.tile([128, 8 * 35], f32, name=f"pskc_{co}_{half}", tag="psA")
                    mm = 0
                    for ci in range(2):
                        for t in range(9):
                            dh, dw = divmod(t, 3)
                            off = b0 * KIMG + dh * WK + dw
                            rhs = (xk_t[ci][:, off:off + 8 * KIMG]
                                   .rearrange("p (b q) -> p b q", q=KIMG)[:, :, :35])
                            nc.tensor.matmul(ps[:].rearrange("p (b q) -> p b q", q=35),
                                             ws_or(wk_t, t, ci, co),
                                             rhs,
                                             start=(mm == 0), stop=(mm == 17))
                            mm += 1
                    # relu(x + shift), compact 35 -> 25 cols per batch
                    nc.scalar.activation(
                        yk_t[co][:, b0 * 25:(b0 + 8) * 25]
                        .rearrange("p (b h q) -> p b h q", h=5, q=5),
                        ps[:].rearrange("p (b h q) -> p b h q", h=5, q=7)[:, :, :, :5],
                        mybir.ActivationFunctionType.Relu,
                        bias=shk_t[co][:, 0:1],
                    )

            # ---- per-batch: search conv + epilogue + xcorr ----
            # search conv (exact 29-wide rows): chunk A rows 0..15 (464 cols),
            # chunk B rows 16..28 (377 cols)
            CHUNKS = ((0, 16), (16, 13))
            for b in range(BL):
                for co in range(2):
                    ys = yspool.tile([128, YSIMG + YS_PAD], f16, name=f"ys_{b}_{co}", tag=f"ys{co}")
                    nc.gpsimd.memset(ys[:, YSIMG:], 0.0)
                    for li, (r0, nrow) in enumerate(CHUNKS):
                        pool, tag = (psa, "psA") if li == 0 else (psb, "psB")
                        ps = pool.tile([128, nrow * WSC], f32, name=f"ps_{b}_{co}_{li}", tag=tag)
                        mm = 0
                        for ci in range(2):
                            for t in range(9):
                                dh, dw = divmod(t, 3)
                                base = b * SIMG + (r0 + dh) * WS + dw
                                rhs = (xs_t[ci][:, base:base + nrow * WS]
                                       .rearrange("p (r q) -> p r q", q=WS)[:, :, :WSC])
                                nc.tensor.matmul(
                                    ps[:].rearrange("p (r q) -> p r q", q=WSC),
                                    ws_or(ws_t, t, ci, co), rhs,
                                    start=(mm == 0), stop=(mm == 17))
                                mm += 1
                        nc.scalar.activation(
                            ys[:, r0 * WSC:(r0 + nrow) * WSC], ps[:],
                            mybir.ActivationFunctionType.Relu,
                            bias=shs_t[co][:, 0:1],
                        )

                    # xcorr: acc[p, n] = sum_t yk[p, b*25+t] * ys[p, n + dy*29+dx]
                    kofs = b * 25
                    taps = [(t,) + divmod(t, 5) for t in range(25)]

                    xps = None
                    if N_PE:
                        # diag matrices for PE taps, one DVE op
                        dg = dgpool.tile([128, N_PE * 128], f16,
                                         name=f"dg_{b}_{co}", tag=f"dg{co}")
                        nc.vector.tensor_tensor(
                            dg[:].rearrange("p (t j) -> p t j", j=128),
                            ident[:].rearrange("p (o j) -> p o j", o=1)
                            .broadcast_to((128, N_PE, 128)),
                            yk_t[co][:, kofs:kofs + N_PE]
                            .rearrange("p (t o) -> p t o", o=1)
                            .broadcast_to((128, N_PE, 128)),
                            mybir.AluOpType.mult)
                        xps = psx.tile([128, ACC_N], f32, name=f"xps_{b}_{co}", tag="psX")
                        for i in range(N_PE):
                            t, dy, dx = taps[i]
                            off = dy * WSC + dx
                            for (c0, cnt) in ((0, 512), (512, ACC_N - 512)):
                                nc.tensor.matmul(
                                    xps[:, c0:c0 + cnt],
                                    dg[:, i * 128:(i + 1) * 128],
                                    ys[:, off + c0:off + c0 + cnt],
                                    start=(i == 0), stop=(i == N_PE - 1))

                    acc = accpool.tile([128, ACC_N], f16, name=f"acc_{b}_{co}", tag=f"acc{co}")

                    # ScalarE products, added into acc by VectorE tensor_tensor
                    tmps = []
                    for j in range(N_AD):
                        t, dy, dx = taps[N_PE + N_DVE + j]
                        off = dy * WSC + dx
                        tmp = tmppool.tile([128, ACC_N], f16,
                                           name=f"tmp{j}_{b}_{co}", tag=f"tmp{j}_{co}")
                        nc.scalar.activation(
                            tmp[:], ys[:, off:off + ACC_N],
                            mybir.ActivationFunctionType.Copy,
                            bias=0.0, scale=yk_t[co][:, kofs + t:kofs + t + 1])
                        tmps.append(tmp)

                    # seed acc from PE psum partial via ScalarE, then
                    # VectorE chain in place
                    if xps is not None:
                        nc.scalar.activation(acc[:], xps[:],
                                             mybir.ActivationFunctionType.Copy)
                    for i in range(N_DVE):
                        t, dy, dx = taps[N_PE + i]
                        off = dy * WSC + dx
                        sl = ys[:, off:off + ACC_N]
                        kcol = yk_t[co][:, kofs + t:kofs + t + 1]
                        if i == 0 and xps is None:
                            nc.vector.tensor_scalar_mul(acc[:], sl, kcol)
                        else:
                            nc.vector.scalar_tensor_tensor(
                                acc[:], sl, kcol, acc[:],
                                mybir.AluOpType.mult, mybir.AluOpType.add)

                    # VectorE accumulates the ScalarE products
                    for tmp in tmps:
                        nc.vector.tensor_tensor(acc[:], tmp[:], acc[:],
                                                mybir.AluOpType.add)

                    nc.sync.dma_start(out=y_d.ap()[co, b], in_=acc[:])

    if not nc.is_finalized():
        nc.finalize()
    return nc


def ws_or(wlist, t, ci, co):
    c0 = (t * 2 + co) * 128
    return wlist[ci][:, c0:c0 + 128]


# ---------------------------------------------------------------------------
# host-side prep + entry point
# ---------------------------------------------------------------------------


def _fold_bn(w, gamma, beta, mean, var):
    # returns folded weight [C,C,3,3] and shift [C] (fp64 math)
    scale = gamma.astype(np.float64) / np.sqrt(var.astype(np.float64) + BN_EPS)
    shift = beta.astype(np.float64) - mean.astype(np.float64) * scale
    wf = w.astype(np.float64) * scale[:, None, None, None]
    return wf, shift


def _w_tiles(wf):
    # wf [C,C,3,3] -> packed [2, 128, 9*2*128] fp16, lhsT[ci, co] layout,
    # column index = (t*2 + co_blk)*128 + co_local
    a = wf.reshape(2, 128, 2, 128, 3, 3)       # [co_b, co_l, ci_b, ci_l, dh, dw]
    a = np.transpose(a, (2, 3, 4, 5, 0, 1))    # [ci_b, ci_l, dh, dw, co_b, co_l]
    return np.ascontiguousarray(a).reshape(2, 128, 9 * 2 * 128).astype(np.float16)


def _make_in_maps(inputs):
    wkf, shk = _fold_bn(inputs["w_kernel"], inputs["bn_k_gamma"],
                        inputs["bn_k_beta"], inputs["bn_k_mean"],
                        inputs["bn_k_var"])
    wsf, shs = _fold_bn(inputs["w_search"], inputs["bn_s_gamma"],
                        inputs["bn_s_beta"], inputs["bn_s_mean"],
                        inputs["bn_s_var"])
    wk_tiles = _w_tiles(wkf)
    ws_tiles = _w_tiles(wsf)
    shk_a = shk.astype(np.float32).reshape(2, 128, 1)
    shs_a = shs.astype(np.float32).reshape(2, 128, 1)

    # [B, C, H, W] -> per-core [2, 128, BL*IMG] fp16 channel-major
    def shard(x, img):
        x = np.asarray(x, np.float16).reshape(NCORES, BL, 2, 128, img)
        return np.ascontiguousarray(x.transpose(0, 2, 3, 1, 4)).reshape(
            NCORES, 2, 128, BL * img)

    xs_sh = shard(inputs["search"], SIMG)
    xk_sh = shard(inputs["kernel"], KIMG)

    ident = np.eye(128, dtype=np.float16)
    return [
        dict(xs=xs_sh[i], xk=xk_sh[i], ws=ws_tiles, wk=wk_tiles,
             shs=shs_a, shk=shk_a, ident=ident)
        for i in range(NCORES)
    ]


def kernel(kernel, search, w_kernel, bn_k_gamma, bn_k_beta, bn_k_mean, bn_k_var,
           w_search, bn_s_gamma, bn_s_beta, bn_s_mean, bn_s_var):
    _install_env_fixups()
    from concourse.bass_utils import run_bass_kernel_spmd

    global _PROGRAM
    if _PROGRAM is None:
        _PROGRAM = _build_program()
    nc = _PROGRAM

    in_maps = _make_in_maps(dict(
        kernel=kernel, search=search, w_kernel=w_kernel,
        bn_k_gamma=bn_k_gamma, bn_k_beta=bn_k_beta, bn_k_mean=bn_k_mean,
        bn_k_var=bn_k_var, w_search=w_search, bn_s_gamma=bn_s_gamma,
        bn_s_beta=bn_s_beta, bn_s_mean=bn_s_mean, bn_s_var=bn_s_var))
    res = run_bass_kernel_spmd(nc, in_maps, list(range(NCORES)))

    out = np.empty((B, C, HO, WO), np.float32)
    for i in range(NCORES):
        y = res.results[i]["y"]  # [2, BL, 128, 725] fp16
        y = y.reshape(2, BL, 128, HO, WSC)[:, :, :, :, :WO].astype(np.float32)
        # -> [BL, 2*128, 25, 25]
        out[i * BL:(i + 1) * BL] = y.transpose(1, 0, 2, 3, 4).reshape(
            BL, C, HO, WO)
    return out


# revision 21
# speedup vs baseline: 1.9884x; 1.0215x over previous
"""DepthwiseXCorr (SiamRPN-style) Trainium2 kernel.

reference:
  k = relu(BN(conv3x3(kernel)))   [B,256,7,7]  -> [B,256,5,5]
  s = relu(BN(conv3x3(search)))   [B,256,31,31]-> [B,256,29,29]
  out[b,c] = valid_xcorr(s[b,c], k[b,c])       -> [B,256,25,25]

Strategy (8 cores, data parallel over batch, 16 batches/core):
  - channel-on-partition layout, fp16 on chip, fp32 PSUM accumulation
  - both convs: 9 shifted matmuls accumulated in PSUM (BN scale folded into
    weights on host), ReLU+shift via ScalarE activation epilogue
  - depthwise xcorr: 25 per-partition-scalar FMA taps
    (scalar_tensor_tensor) split across VectorE and GpSimd
  - output written as fp16 [blk, b, c, 25x29(padded)]; host crops + casts
"""

import sys
import types
import contextlib
import numpy as np

C = 256
B = 128
NCORES = 8
BL = B // NCORES  # 16 local batches
BN_EPS = 1e-5

HS, WS = 31, 31          # search input
HK, WK = 7, 7            # kernel input
HSC, WSC = 29, 29        # search conv output (valid)
HKC, WKC = 5, 5          # kernel conv output (valid)
HO, WO = 25, 25          # xcorr output (valid)

SIMG = HS * WS           # 961
KIMG = HK * WK           # 49
YSIMG = HSC * WSC        # 841
NPAD_S = 64              # tail pad for shifted reads
NPAD_K = 64
YS_PAD = 8               # ys tile tail pad (max tap shift 4*29+4 + 725 = 845)
ACC_N = HO * WSC         # 725 = 25 rows x 29 cols (padded width)

# tap split across engines (tap index t = dy*5+dx):
# (n_pe, n_dve, n_ad): TensorE full-diag / VectorE STT / ScalarE-product+VectorE-add
def tap_split(b):
    return (19, 3, 3) if b >= BL - 2 else (12, 4, 9)


# ---------------------------------------------------------------------------
# environment fixups
# ---------------------------------------------------------------------------

_FIXED = False


def _install_env_fixups():
    global _FIXED
    if _FIXED:
        return
    _FIXED = True

    # -- 1. walrus in this image rejects >1 sync-wait on the Tile tail drain;
    #       spread waits over single-wait SP nops.
    import concourse.tile as tile_mod
    from concourse import mybir
    from concourse.vector_clock import ScopedClock

    def _patched_drain_and_barrier(self, tick_clock, wait_clock):
        nc = self.nc
        probe = nc.sync.nop(hint="drain_wait_spread", nofuse=True)
        wait_clock.add_sem_waits(
            probe.ins, ScopedClock({None: tick_clock.global_clock})
        )
        si = probe.ins.sync_info
        waits = list(si.on_wait) if si is not None and si.on_wait else []
        if len(waits) > 1:
            probe.ins.sync_info = mybir.SyncInfo(on_wait=waits[:1], on_update=[])
            for w in waits[1:]:
                n = nc.sync.nop(hint="drain_wait_spread", nofuse=True)
                n.ins.sync_info = mybir.SyncInfo(on_wait=[w], on_update=[])
        nc.sync.drain()
        nc.all_engine_barrier()
        assert self.sems is not None
        popped = nc._tile_sem_poison_stack.pop()
        assert popped is self._sem_poison
        nc.clear_and_free_semaphores(list(self.sems.allocated().values()))
        nc.all_engine_barrier()

    tile_mod.TileContext._drain_and_barrier = _patched_drain_and_barrier

    # -- 1b. optionally re-enable walrus LDWEIGHTS scheduling optimization
    #        (hides weight loads under matmul streaming).
    import os as _os

    if _os.environ.get("KERNEL_LDW_OPT", "1") == "1":
        import concourse.bass_utils as _bu

        _orig_bvo = _bu.bir_verify_and_optimise

        def _bvo(*args, **kwargs):
            import subprocess as _sp

            orig_run = _bu.run_command

            def run_command(cmd, **kw):
                cmd = [c.replace("--enable-ldw-opt=false", "--enable-ldw-opt=true")
                       if isinstance(c, str) else c for c in cmd]
                return orig_run(cmd, **kw)

            _bu.run_command = run_command
            try:
                return _orig_bvo(*args, **kwargs)
            finally:
                _bu.run_command = orig_run

        _bu.bir_verify_and_optimise = _bvo

    # -- 2. antenv.axon_hooks shim (NTFF profiling hook) so trace=True works.
    try:
        import antenv.axon_hooks  # noqa: F401
    except ImportError:
        import ctypes

        mod = types.ModuleType("antenv.axon_hooks")
        _holder = {}

        def set_axon_ntff_profile_hook(h):
            _holder["h"] = h

        def _default_hook():
            so_path = "/opt/axon/libaxon_pjrt.so"
            try:
                lib = ctypes.CDLL(so_path)
            except OSError:
                return None
            if not hasattr(lib, "axon_start_nrt_profile"):
                return None
            lib.axon_start_nrt_profile.argtypes = [
                ctypes.POINTER(ctypes.c_int64),
                ctypes.c_size_t,
            ]
            lib.axon_start_nrt_profile.restype = ctypes.c_int64
            lib.axon_stop_nrt_profile.argtypes = [ctypes.c_char_p]
            lib.axon_stop_nrt_profile.restype = ctypes.c_int64

            @contextlib.contextmanager
            def _hook(output_dir, device_ids):
                import jax

                jax.devices()
                if device_ids:
                    ids = (ctypes.c_int64 * len(device_ids))(*device_ids)
                    rc = lib.axon_start_nrt_profile(ids, len(device_ids))
                else:
                    rc = lib.axon_start_nrt_profile(None, 0)
                if rc != 0:
                    raise RuntimeError(f"axon_start_nrt_profile rc={rc}")
                try:
                    yield
                finally:
                    n = lib.axon_stop_nrt_profile(str(output_dir).encode())
                    if n < 0:
                        raise RuntimeError(f"axon_stop_nrt_profile rc={n}")

            return _hook

        def get_axon_ntff_profile_hook():
            if "h" in _holder:
                return _holder["h"]
            return _default_hook()

        mod.set_axon_ntff_profile_hook = set_axon_ntff_profile_hook
        mod.get_axon_ntff_profile_hook = get_axon_ntff_profile_hook
        sys.modules["antenv.axon_hooks"] = mod


# ---------------------------------------------------------------------------
# bass program
# ---------------------------------------------------------------------------

_PROGRAM = None


def _build_program():
    import concourse.tile as tile
    from concourse import bacc, mybir

    f16 = mybir.dt.float16
    f32 = mybir.dt.float32

    nc = bacc.Bacc("TRN2", target_bir_lowering=False, debug=False)

    xs_d = nc.dram_tensor("xs", [2, 128, BL * SIMG], f16, kind="ExternalInput")
    xk_d = nc.dram_tensor("xk", [2, 128, BL * KIMG], f16, kind="ExternalInput")
    ws_d = nc.dram_tensor("ws", [2, 128, 9 * 2 * 128], f16, kind="ExternalInput")
    wk_d = nc.dram_tensor("wk", [2, 128, 9 * 2 * 128], f16, kind="ExternalInput")
    shs_d = nc.dram_tensor("shs", [2, 128, 1], f32, kind="ExternalInput")
    shk_d = nc.dram_tensor("shk", [2, 128, 1], f32, kind="ExternalInput")
    id_d = nc.dram_tensor("ident", [128, 128], f16, kind="ExternalInput")
    y_d = nc.dram_tensor("y", [2, BL, 128, ACC_N], f16, kind="ExternalOutput")

    with tile.TileContext(nc) as tc:
        with (
            tc.tile_pool(name="wpool", bufs=1) as wpool,
            tc.tile_pool(name="xpool", bufs=1) as xpool,
            tc.tile_pool(name="shpool", bufs=1) as shpool,
            tc.tile_pool(name="ykpool", bufs=1) as ykpool,
            tc.tile_pool(name="yspool", bufs=5) as yspool,
            tc.tile_pool(name="accpool", bufs=6) as accpool,
            tc.tile_pool(name="dgpool", bufs=3) as dgpool,
            tc.tile_pool(name="tmppool", bufs=2) as tmppool,
            tc.tile_pool(name="psa", bufs=2, space="PSUM") as psa,
            tc.tile_pool(name="psb", bufs=2, space="PSUM") as psb,
            tc.tile_pool(name="psx", bufs=2, space="PSUM") as psx,
        ):
            # ---- static input loads (weights packed: 1 DMA per tensor/block) ----
            ws_t = [wpool.tile([128, 9 * 2 * 128], f16, name=f"wsp{ci}", tag=f"wsp{ci}")
                    for ci in range(2)]
            wk_t = [wpool.tile([128, 9 * 2 * 128], f16, name=f"wkp{ci}", tag=f"wkp{ci}")
                    for ci in range(2)]
            for ci in range(2):
                nc.sync.dma_start(out=ws_t[ci][:], in_=ws_d.ap()[ci])
                nc.sync.dma_start(out=wk_t[ci][:], in_=wk_d.ap()[ci])

            shs_t = [shpool.tile([128, 1], f32, name=f"shs{co}", tag=f"shs{co}") for co in range(2)]
            shk_t = [shpool.tile([128, 1], f32, name=f"shk{co}", tag=f"shk{co}") for co in range(2)]
            for co in range(2):
                nc.sync.dma_start(out=shs_t[co][:], in_=shs_d.ap()[co])
                nc.sync.dma_start(out=shk_t[co][:], in_=shk_d.ap()[co])

            xs_t = [xpool.tile([128, BL * SIMG + NPAD_S], f16, name=f"xs{ci}", tag=f"xs{ci}")
                    for ci in range(2)]
            xk_t = [xpool.tile([128, BL * KIMG + NPAD_K], f16, name=f"xk{ci}", tag=f"xk{ci}")
                    for ci in range(2)]
            for ci in range(2):
                for b in range(BL):
                    nc.sync.dma_start(
                        out=xs_t[ci][:, b * SIMG:(b + 1) * SIMG],
                        in_=xs_d.ap()[ci][:, b * SIMG:(b + 1) * SIMG])
                nc.gpsimd.memset(xs_t[ci][:, BL * SIMG:], 0.0)
                nc.sync.dma_start(out=xk_t[ci][:, :BL * KIMG], in_=xk_d.ap()[ci])
                nc.gpsimd.memset(xk_t[ci][:, BL * KIMG:], 0.0)

            ident = shpool.tile([128, 128], f16, name="ident", tag="ident")
            nc.sync.dma_start(out=ident[:], in_=id_d.ap())

            # ---- kernel-branch conv: [2,128, BL*49] -> yk [2][128, BL*25] ----
            yk_t = [ykpool.tile([128, BL * 25], f32, name=f"yk{co}", tag=f"yk{co}") for co in range(2)]
            for co in range(2):
                for half in range(2):  # batches 0..7, 8..15
                    b0 = half * 8
                    ps = psa.tile([128, 8 * 35], f32, name=f"pskc_{co}_{half}", tag="psA")
                    mm = 0
                    for ci in range(2):
                        for t in range(9):
                            dh, dw = divmod(t, 3)
                            off = b0 * KIMG + dh * WK + dw
                            rhs = (xk_t[ci][:, off:off + 8 * KIMG]
                                   .rearrange("p (b q) -> p b q", q=KIMG)[:, :, :35])
                            nc.tensor.matmul(ps[:].rearrange("p (b q) -> p b q", q=35),
                                             ws_or(wk_t, t, ci, co),
                                             rhs,
                                             start=(mm == 0), stop=(mm == 17))
                            mm += 1
                    # relu(x + shift), compact 35 -> 25 cols per batch
                    nc.scalar.activation(
                        yk_t[co][:, b0 * 25:(b0 + 8) * 25]
                        .rearrange("p (b h q) -> p b h q", h=5, q=5),
                        ps[:].rearrange("p (b h q) -> p b h q", h=5, q=7)[:, :, :, :5],
                        mybir.ActivationFunctionType.Relu,
                        bias=shk_t[co][:, 0:1],
                    )

            # ---- per-batch: search conv + epilogue + xcorr ----
            # search conv (exact 29-wide rows): chunk A rows 0..15 (464 cols),
            # chunk B rows 16..28 (377 cols)
            CHUNKS = ((0, 16), (16, 13))
            for b in range(BL):
                for co in range(2):
                    ys = yspool.tile([128, YSIMG + YS_PAD], f16, name=f"ys_{b}_{co}", tag=f"ys{co}")
                    nc.gpsimd.memset(ys[:, YSIMG:], 0.0)
                    for li, (r0, nrow) in enumerate(CHUNKS):
                        pool, tag = (psa, "psA") if li == 0 else (psb, "psB")
                        ps = pool.tile([128, nrow * WSC], f32, name=f"ps_{b}_{co}_{li}", tag=tag)
                        mm = 0
                        for ci in range(2):
                            for t in range(9):
                                dh, dw = divmod(t, 3)
                                base = b * SIMG + (r0 + dh) * WS + dw
                                rhs = (xs_t[ci][:, base:base + nrow * WS]
                                       .rearrange("p (r q) -> p r q", q=WS)[:, :, :WSC])
                                nc.tensor.matmul(
                                    ps[:].rearrange("p (r q) -> p r q", q=WSC),
                                    ws_or(ws_t, t, ci, co), rhs,
                                    start=(mm == 0), stop=(mm == 17))
                                mm += 1
                        nc.scalar.activation(
                            ys[:, r0 * WSC:(r0 + nrow) * WSC], ps[:],
                            mybir.ActivationFunctionType.Relu,
                            bias=shs_t[co][:, 0:1],
                        )

                    # xcorr: acc[p, n] = sum_t yk[p, b*25+t] * ys[p, n + dy*29+dx]
                    kofs = b * 25
                    taps = [(t,) + divmod(t, 5) for t in range(25)]
                    N_PE, N_DVE, N_AD = tap_split(b)

                    xps = None
                    if N_PE:
                        # diag matrices for PE taps, one DVE op
                        dg = dgpool.tile([128, N_PE * 128], f16,
                                         name=f"dg_{b}_{co}", tag=f"dg{co}")
                        nc.vector.tensor_tensor(
                            dg[:].rearrange("p (t j) -> p t j", j=128),
                            ident[:].rearrange("p (o j) -> p o j", o=1)
                            .broadcast_to((128, N_PE, 128)),
                            yk_t[co][:, kofs:kofs + N_PE]
                            .rearrange("p (t o) -> p t o", o=1)
                            .broadcast_to((128, N_PE, 128)),
                            mybir.AluOpType.mult)
                        xps = psx.tile([128, ACC_N], f32, name=f"xps_{b}_{co}", tag="psX")
                        for i in range(N_PE):
                            t, dy, dx = taps[i]
                            off = dy * WSC + dx
                            for (c0, cnt) in ((0, 512), (512, ACC_N - 512)):
                                nc.tensor.matmul(
                                    xps[:, c0:c0 + cnt],
                                    dg[:, i * 128:(i + 1) * 128],
                                    ys[:, off + c0:off + c0 + cnt],
                                    start=(i == 0), stop=(i == N_PE - 1))

                    acc = accpool.tile([128, ACC_N], f16, name=f"acc_{b}_{co}", tag=f"acc{co}")

                    # ScalarE products, added into acc by VectorE tensor_tensor
                    tmps = []
                    for j in range(N_AD):
                        t, dy, dx = taps[N_PE + N_DVE + j]
                        off = dy * WSC + dx
                        tmp = tmppool.tile([128, ACC_N], f16,
                                           name=f"tmp{j}_{b}_{co}", tag=f"tmp{j}_{co}")
                        nc.scalar.activation(
                            tmp[:], ys[:, off:off + ACC_N],
                            mybir.ActivationFunctionType.Copy,
                            bias=0.0, scale=yk_t[co][:, kofs + t:kofs + t + 1])
                        tmps.append(tmp)

                    # seed acc from PE psum partial via ScalarE, then
                    # VectorE chain in place
                    if xps is not None:
                        nc.scalar.activation(acc[:], xps[:],
                                             mybir.ActivationFunctionType.Copy)
                    for i in range(N_DVE):
                        t, dy, dx = taps[N_PE + i]
                        off = dy * WSC + dx
                        sl = ys[:, off:off + ACC_N]
                        kcol = yk_t[co][:, kofs + t:kofs + t + 1]
                        if i == 0 and xps is None:
                            nc.vector.tensor_scalar_mul(acc[:], sl, kcol)
                        else:
                            nc.vector.scalar_tensor_tensor(
                                acc[:], sl, kcol, acc[:],
                                mybir.AluOpType.mult, mybir.AluOpType.add)

                    # VectorE accumulates the ScalarE products
                    for tmp in tmps:
                        nc.vector.tensor_tensor(acc[:], tmp[:], acc[:],
                                                mybir.AluOpType.add)

                    nc.sync.dma_start(out=y_d.ap()[co, b], in_=acc[:])

    if not nc.is_finalized():
        nc.finalize()
    return nc


def ws_or(wlist, t, ci, co):
    c0 = (t * 2 + co) * 128
    return wlist[ci][:, c0:c0 + 128]


# ---------------------------------------------------------------------------
# host-side prep + entry point
# ---------------------------------------------------------------------------


def _fold_bn(w, gamma, beta, mean, var):
    # returns folded weight [C,C,3,3] and shift [C] (fp64 math)
    scale = gamma.astype(np.float64) / np.sqrt(var.astype(np.float64) + BN_EPS)
    shift = beta.astype(np.float64) - mean.astype(np.float64) * scale
    wf = w.astype(np.float64) * scale[:, None, None, None]
    return wf, shift


def _w_tiles(wf):
    # wf [C,C,3,3] -> packed [2, 128, 9*2*128] fp16, lhsT[ci, co] layout,
    # column index = (t*2 + co_blk)*128 + co_local
    a = wf.reshape(2, 128, 2, 128, 3, 3)       # [co_b, co_l, ci_b, ci_l, dh, dw]
    a = np.transpose(a, (2, 3, 4, 5, 0, 1))    # [ci_b, ci_l, dh, dw, co_b, co_l]
    return np.ascontiguousarray(a).reshape(2, 128, 9 * 2 * 128).astype(np.float16)


def _make_in_maps(inputs):
    wkf, shk = _fold_bn(inputs["w_kernel"], inputs["bn_k_gamma"],
                        inputs["bn_k_beta"], inputs["bn_k_mean"],
                        inputs["bn_k_var"])
    wsf, shs = _fold_bn(inputs["w_search"], inputs["bn_s_gamma"],
                        inputs["bn_s_beta"], inputs["bn_s_mean"],
                        inputs["bn_s_var"])
    wk_tiles = _w_tiles(wkf)
    ws_tiles = _w_tiles(wsf)
    shk_a = shk.astype(np.float32).reshape(2, 128, 1)
    shs_a = shs.astype(np.float32).reshape(2, 128, 1)

    # [B, C, H, W] -> per-core [2, 128, BL*IMG] fp16 channel-major
    def shard(x, img):
        x = np.asarray(x, np.float16).reshape(NCORES, BL, 2, 128, img)
        return np.ascontiguousarray(x.transpose(0, 2, 3, 1, 4)).reshape(
            NCORES, 2, 128, BL * img)

    xs_sh = shard(inputs["search"], SIMG)
    xk_sh = shard(inputs["kernel"], KIMG)

    ident = np.eye(128, dtype=np.float16)
    return [
        dict(xs=xs_sh[i], xk=xk_sh[i], ws=ws_tiles, wk=wk_tiles,
             shs=shs_a, shk=shk_a, ident=ident)
        for i in range(NCORES)
    ]


def kernel(kernel, search, w_kernel, bn_k_gamma, bn_k_beta, bn_k_mean, bn_k_var,
           w_search, bn_s_gamma, bn_s_beta, bn_s_mean, bn_s_var):
    _install_env_fixups()
    from concourse.bass_utils import run_bass_kernel_spmd

    global _PROGRAM
    if _PROGRAM is None:
        _PROGRAM = _build_program()
    nc = _PROGRAM

    in_maps = _make_in_maps(dict(
        kernel=kernel, search=search, w_kernel=w_kernel,
        bn_k_gamma=bn_k_gamma, bn_k_beta=bn_k_beta, bn_k_mean=bn_k_mean,
        bn_k_var=bn_k_var, w_search=w_search, bn_s_gamma=bn_s_gamma,
        bn_s_beta=bn_s_beta, bn_s_mean=bn_s_mean, bn_s_var=bn_s_var))
    res = run_bass_kernel_spmd(nc, in_maps, list(range(NCORES)))

    out = np.empty((B, C, HO, WO), np.float32)
    for i in range(NCORES):
        y = res.results[i]["y"]  # [2, BL, 128, 725] fp16
        y = y.reshape(2, BL, 128, HO, WSC)[:, :, :, :, :WO].astype(np.float32)
        # -> [BL, 2*128, 25, 25]
        out[i * BL:(i + 1) * BL] = y.transpose(1, 0, 2, 3, 4).reshape(
            BL, C, HO, WO)
    return out
